# revision 16
# baseline (speedup 1.0000x reference)
"""MGU (minimal gated unit) RNN kernel for Trainium2, 8-core data-parallel.

Problem (hardcoded):
    inputs [64, 512, 256] f32, Wf/Ws [768, 512], bf/bs [512], Wo [512, 256], bo [256]
    h_t = (1-f)*h + f*tanh(([f*h, x] @ Ws) + bs),  f = sigmoid(([h, x] @ Wf) + bf)
    outputs = relu(all_hidden @ Wo + bo);  returns (outputs, final_state)

Sharding: batch 64 -> 8 cores x 8. Weights replicated. Scan local per core.

Per-core layout convention: "transposed-land".
    hidden index h = jc*128 + p   (p = partition, jc in 0..3)
    token index tok = t*8 + b     (time-major within the core's batch slice)
All gate tensors live as [128, 4, 8] tiles (partition, hidden-chunk, batch).
Within a chunk, token columns are ordered q-subtile-major then batch-major:
    col = q*128 + b*16 + t_local   (q = t//16 within chunk, t_local = t%16)
so x DMAs use the adjacent "(b t)" grouping and per-step slices are strided.
Scan matmuls: out[j, b] += Wh[k, j].T-style with W as the stationary operand:
    matmul(psum[:, jc, :], lhsT=Wh[:, kc, jc*128:+128], rhs=h_prev[:, kc, :])
Input/output projections are wide-N matmuls over 512-token chunks.
"""

import os
import sys
import types
from contextlib import ExitStack

import numpy as np

# The agent image's `antenv` package lacks `axon_hooks`, which degrades NTFF
# profiling (trace=True) to a hard ImportError in bass_utils. Pre-seed a
# minimal implementation; trn_boot's set_axon_ntff_profile_hook call then
# registers the real ctypes-driven hook at jax boot time.
if "antenv.axon_hooks" not in sys.modules:
    _m = types.ModuleType("antenv.axon_hooks")
    _m._hook = None

    def _set_hook(h, _m=_m):
        _m._hook = h

    def _get_hook(_m=_m):
        return _m._hook

    _m.set_axon_ntff_profile_hook = _set_hook
    _m.get_axon_ntff_profile_hook = _get_hook
    sys.modules["antenv.axon_hooks"] = _m
    try:
        from trn_agent_boot.trn_boot import _ntff_profile_via_ctypes

        _h = _ntff_profile_via_ctypes("/opt/axon/libaxon_pjrt.so")
        if _h is not None:
            _set_hook(_h)
    except Exception:
        pass

import concourse.bacc as bacc
import concourse.mybir as mybir
from concourse import masks, tile

P = 128
B = 8          # batch per core
S = 512        # sequence length
IN = 256       # input units
H = 512        # hidden units
O = 256        # output units
HC = H // P    # hidden chunks (4)
ICC = IN // P  # input chunks (2)
N_CORES = 8
F32 = mybir.dt.float32


def build(seq_len=S, debug=False):
    nc = bacc.Bacc(None, target_bir_lowering=False, debug=debug)
    AF = mybir.ActivationFunctionType

    TC = min(64, seq_len)          # timesteps per chunk
    NCH = seq_len // TC            # chunks
    TOK = TC * B                   # tokens per chunk (<= 512)
    QN = TOK // P                  # 128-token subtiles per chunk

    x_d = nc.dram_tensor("x", [B, seq_len, IN], F32, kind="ExternalInput")
    wf_d = nc.dram_tensor("Wf", [H + IN, H], F32, kind="ExternalInput")
    bf_d = nc.dram_tensor("bf", [H], F32, kind="ExternalInput")
    ws_d = nc.dram_tensor("Ws", [H + IN, H], F32, kind="ExternalInput")
    bs_d = nc.dram_tensor("bs", [H], F32, kind="ExternalInput")
    wo_d = nc.dram_tensor("Wo", [H, O], F32, kind="ExternalInput")
    bo_d = nc.dram_tensor("bo", [O], F32, kind="ExternalInput")
    out_d = nc.dram_tensor("out", [B, seq_len, O], F32, kind="ExternalOutput")
    hfin_d = nc.dram_tensor("hfin", [B, H], F32, kind="ExternalOutput")

    with tile.TileContext(nc) as tc, ExitStack() as ctx:
        dma = nc.sync.dma_start
        cpool = ctx.enter_context(tc.tile_pool(name="const", bufs=1))

        # --- constants / weights in SBUF ---
        wfh = cpool.tile([P, HC, H], F32)   # Wf rows 0:512   (h part)
        wsh = cpool.tile([P, HC, H], F32)
        wfx = cpool.tile([P, ICC, H], F32)  # Wf rows 512:768 (x part)
        wsx = cpool.tile([P, ICC, H], F32)
        wo_sb = cpool.tile([P, HC, O], F32)
        bft = cpool.tile([P, HC], F32)
        bst = cpool.tile([P, HC], F32)
        ident = cpool.tile([P, P], F32)
        h0 = cpool.tile([P, HC, B], F32)
        ones = cpool.tile([1, P], F32)
        bo_row = cpool.tile([1, O], F32)
        bo_bc = cpool.tile([P, O], F32)
        hfin_sb = cpool.tile([B, H], F32)

        dma(wfh[:], wf_d[0:H, :].rearrange("(kc p) j -> p kc j", p=P))
        dma(wsh[:], ws_d[0:H, :].rearrange("(kc p) j -> p kc j", p=P))
        dma(wfx[:], wf_d[H:H + IN, :].rearrange("(kc p) j -> p kc j", p=P))
        dma(wsx[:], ws_d[H:H + IN, :].rearrange("(kc p) j -> p kc j", p=P))
        dma(wo_sb[:], wo_d.rearrange("(kc p) o -> p kc o", p=P))
        dma(bft[:], bf_d.rearrange("(jc p) -> p jc", p=P))
        dma(bst[:], bs_d.rearrange("(jc p) -> p jc", p=P))
        dma(bo_row[:], bo_d[None, :])
        masks.make_identity(nc, ident[:])
        nc.vector.memset(ones[:], 1.0)
        nc.vector.memset(h0[:], 0.0)

        # psum pools (8 banks total: 2+2+1+1+2)
        tp_pool = ctx.enter_context(tc.tile_pool(name="tp", bufs=2, space="PSUM"))
        p1_pool = ctx.enter_context(tc.tile_pool(name="p1", bufs=2, space="PSUM"))
        pf_pool = ctx.enter_context(tc.tile_pool(name="pf", bufs=1, space="PSUM"))
        ps_pool = ctx.enter_context(tc.tile_pool(name="ps", bufs=1, space="PSUM"))
        po_pool = ctx.enter_context(tc.tile_pool(name="po", bufs=2, space="PSUM"))

        # bo broadcast across partitions via ones-matmul
        pbo = po_pool.tile([P, O], F32, tag="po")
        nc.tensor.matmul(pbo[:], ones[:], bo_row[:], start=True, stop=True)
        nc.vector.tensor_copy(bo_bc[:], pbo[:])

        # sbuf pools
        xin_pool = ctx.enter_context(tc.tile_pool(name="xin", bufs=2 * QN))
        xt_pool = ctx.enter_context(tc.tile_pool(name="xt", bufs=2))
        xf_pool = ctx.enter_context(tc.tile_pool(name="xf", bufs=2))
        xs_pool = ctx.enter_context(tc.tile_pool(name="xs", bufs=2))
        ht_pool = ctx.enter_context(tc.tile_pool(name="ht", bufs=2))
        sc_pool = ctx.enter_context(tc.tile_pool(name="sc", bufs=3))
        ob_pool = ctx.enter_context(tc.tile_pool(name="ob", bufs=3))

        def phase1(c):
            """Input projections for chunk c -> (XfT, XsT) [P, HC, TOK]."""
            xins = []
            for q in range(QN):
                xi = xin_pool.tile([P, IN], F32, tag="xin")
                t0 = c * TC + q * (P // B)
                t1 = c * TC + (q + 1) * (P // B)
                TL = P // B
                for b in range(B):
                    dma(xi[b * TL:(b + 1) * TL, :], x_d[b, t0:t1, :])
                xins.append(xi)
            xtt = xt_pool.tile([P, ICC, TOK], F32, tag="xt")
            for q in range(QN):
                for kc in range(ICC):
                    tpt = tp_pool.tile([P, P], F32, tag="tp")
                    nc.tensor.transpose(tpt[:], xins[q][:, kc * P:(kc + 1) * P], ident[:])
                    nc.scalar.copy(xtt[:, kc, q * P:(q + 1) * P], tpt[:])
            xft = xf_pool.tile([P, HC, TOK], F32, tag="xf")
            xst = xs_pool.tile([P, HC, TOK], F32, tag="xs")
            for w_sb, b_t, dst in ((wfx, bft, xft), (wsx, bst, xst)):
                for jc in range(HC):
                    p1t = p1_pool.tile([P, TOK], F32, tag="p1")
                    for kc in range(ICC):
                        nc.tensor.matmul(
                            p1t[:], w_sb[:, kc, jc * P:(jc + 1) * P], xtt[:, kc, :],
                            start=(kc == 0), stop=(kc == ICC - 1))
                    nc.scalar.activation(dst[:, jc, :], p1t[:], AF.Identity,
                                         bias=b_t[:, jc:jc + 1], scale=1.0)
            return xft, xst

        def scan(c, xft, xst, hprev):
            """64 recurrent steps for chunk c; returns HT tile + new hprev."""
            htt = ht_pool.tile([P, HC, TOK], F32, tag="ht")
            TL = P // B  # t_locals per 128-token subtile (16)
            htt_v = htt[:].rearrange("p h (q b t) -> p h q t b", b=B, t=TL)
            xft_v = xft[:].rearrange("p h (q b t) -> p h q t b", b=B, t=TL)
            xst_v = xst[:].rearrange("p h (q b t) -> p h q t b", b=B, t=TL)
            for t in range(TC):
                q, tl = t // TL, t % TL
                pf_t = pf_pool.tile([P, HC, B], F32, tag="pf")
                for jc in range(HC):
                    for kc in range(HC):
                        nc.tensor.matmul(
                            pf_t[:, jc, :], wfh[:, kc, jc * P:(jc + 1) * P],
                            hprev[:, kc, :], start=(kc == 0), stop=(kc == HC - 1))
                zf = sc_pool.tile([P, HC, B], F32, tag="zf")
                nc.vector.tensor_add(zf[:], pf_t[:], xft_v[:, :, q, tl, :])
                f = sc_pool.tile([P, HC, B], F32, tag="f")
                nc.scalar.activation(f[:], zf[:], AF.Sigmoid)
                g = sc_pool.tile([P, HC, B], F32, tag="g")
                nc.vector.tensor_mul(g[:], f[:], hprev[:])
                ps_t = ps_pool.tile([P, HC, B], F32, tag="ps")
                for jc in range(HC):
                    for kc in range(HC):
                        nc.tensor.matmul(
                            ps_t[:, jc, :], wsh[:, kc, jc * P:(jc + 1) * P],
                            g[:, kc, :], start=(kc == 0), stop=(kc == HC - 1))
                zs = sc_pool.tile([P, HC, B], F32, tag="zs")
                nc.vector.tensor_add(zs[:], ps_t[:], xst_v[:, :, q, tl, :])
                s = sc_pool.tile([P, HC, B], F32, tag="s")
                nc.scalar.activation(s[:], zs[:], AF.Tanh)
                d = sc_pool.tile([P, HC, B], F32, tag="d")
                nc.vector.tensor_sub(d[:], s[:], hprev[:])
                m = sc_pool.tile([P, HC, B], F32, tag="m")
                nc.vector.tensor_mul(m[:], f[:], d[:])
                hnew = htt_v[:, :, q, tl, :]
                nc.vector.tensor_add(hnew, hprev[:], m[:])
                hprev = hnew
            return htt, hprev

        def phase3(c, htt):
            """Output projection + relu for chunk c, DMA to out."""
            for q in range(QN):
                pot = po_pool.tile([P, O], F32, tag="po")
                for kc in range(HC):
                    nc.tensor.matmul(pot[:], htt[:, kc, q * P:(q + 1) * P],
                                     wo_sb[:, kc, :], start=(kc == 0),
                                     stop=(kc == HC - 1))
                ob = ob_pool.tile([P, O], F32, tag="ob")
                nc.vector.tensor_add(ob[:], pot[:], bo_bc[:])
                ob2 = ob_pool.tile([P, O], F32, tag="ob2")
                nc.vector.tensor_scalar_max(ob2[:], ob[:], 0.0)
                t0 = c * TC + q * (P // B)
                t1 = c * TC + (q + 1) * (P // B)
                TL = P // B
                for b in range(B):
                    dma(out_d[b, t0:t1, :], ob2[b * TL:(b + 1) * TL, :])

        hprev = h0[:, :, :]
        xft, xst = phase1(0)
        for c in range(NCH):
            htt, hprev = scan(c, xft, xst, hprev)
            phase3(c, htt)
            if c + 1 < NCH:
                xft, xst = phase1(c + 1)

        # final state: transpose h_last [P, HC, B] -> [B, H]
        for kc in range(HC):
            tpt = tp_pool.tile([P, P], F32, tag="tp")
            nc.tensor.transpose(tpt[:B, :], hprev[:, kc, :], ident[:])
            nc.vector.tensor_copy(hfin_sb[:, kc * P:(kc + 1) * P], tpt[:B, :])
        dma(hfin_d[:, :], hfin_sb[:])

    nc.compile()
    return nc


_NC_CACHE = {}


def _get_nc():
    if "nc" not in _NC_CACHE:
        _NC_CACHE["nc"] = build()
    return _NC_CACHE["nc"]


def kernel(inputs, Wf, bf, Ws, bs, Wo, bo):
    from concourse.bass_utils import run_bass_kernel_spmd

    inputs = np.ascontiguousarray(np.asarray(inputs, dtype=np.float32))
    Wf = np.ascontiguousarray(np.asarray(Wf, dtype=np.float32))
    bf = np.ascontiguousarray(np.asarray(bf, dtype=np.float32))
    Ws = np.ascontiguousarray(np.asarray(Ws, dtype=np.float32))
    bs = np.ascontiguousarray(np.asarray(bs, dtype=np.float32))
    Wo = np.ascontiguousarray(np.asarray(Wo, dtype=np.float32))
    bo = np.ascontiguousarray(np.asarray(bo, dtype=np.float32))

    nc = _get_nc()
    batch = inputs.shape[0]
    bpc = batch // N_CORES
    in_maps = []
    for i in range(N_CORES):
        in_maps.append({
            "x": inputs[i * bpc:(i + 1) * bpc],
            "Wf": Wf, "bf": bf, "Ws": Ws, "bs": bs, "Wo": Wo, "bo": bo,
        })
    res = run_bass_kernel_spmd(
        nc, in_maps, core_ids=list(range(N_CORES)),
        trace=bool(int(os.environ.get("KERNEL_TRACE", "0"))),
    )
    outs = np.concatenate([r["out"] for r in res.results], axis=0)
    hfin = np.concatenate([r["hfin"] for r in res.results], axis=0)
    _NC_CACHE["last_results"] = res
    return outs, hfin


# revision 19
# speedup vs baseline: 4.3584x; 4.3584x over previous
"""MGU (minimal gated unit) RNN kernel for Trainium2, 8-core data-parallel.

Problem (hardcoded):
    inputs [64, 512, 256] f32, Wf/Ws [768, 512], bf/bs [512], Wo [512, 256], bo [256]
    h_t = (1-f)*h + f*tanh(([f*h, x] @ Ws) + bs),  f = sigmoid(([h, x] @ Wf) + bf)
    outputs = relu(all_hidden @ Wo + bo);  returns (outputs, final_state)

Sharding: batch 64 -> 8 cores x 8. Weights replicated. Scan local per core.

Layout: "transposed-land" — hidden on partitions, (time, batch) on the free dim.
    hidden index h = jc*128 + p; within a 64-step chunk, token col = q*128 + b*16 + t
Scan matmuls are weights-stationary bf16 (fp32 runs as a 4-cycle/row double
pass on this PE; bf16 LDW+MM pairs measure ~42ns vs fp32 ~435ns):
    matmul(psum[:, jc, :], lhsT=Wh_bf[:, kc, jc*128:+128], rhs=h_bf[:, kc, :])
Gate math runs fp32 from PSUM; the hidden state is stored bf16 (matmul rhs +
output-projection stationary). Input/output projections are wide-N bf16
matmuls per 512-token chunk, fully fused (no DRAM scratch).
"""

import os
import sys
import types
from contextlib import ExitStack

import numpy as np

# The agent image's `antenv` package lacks `axon_hooks`, which degrades NTFF
# profiling (trace=True) to a hard ImportError in bass_utils. Pre-seed a
# minimal implementation and register the ctypes-driven hook.
if "antenv.axon_hooks" not in sys.modules:
    _m = types.ModuleType("antenv.axon_hooks")
    _m._hook = None

    def _set_hook(h, _m=_m):
        _m._hook = h

    def _get_hook(_m=_m):
        return _m._hook

    _m.set_axon_ntff_profile_hook = _set_hook
    _m.get_axon_ntff_profile_hook = _get_hook
    sys.modules["antenv.axon_hooks"] = _m
    try:
        from trn_agent_boot.trn_boot import _ntff_profile_via_ctypes

        _h = _ntff_profile_via_ctypes("/opt/axon/libaxon_pjrt.so")
        if _h is not None:
            _set_hook(_h)
    except Exception:
        pass

import concourse.bacc as bacc
import concourse.mybir as mybir
from concourse import masks, tile

P = 128
B = 8          # batch per core
S = 512        # sequence length
IN = 256       # input units
H = 512        # hidden units
O = 256        # output units
HC = H // P    # hidden chunks (4)
ICC = IN // P  # input chunks (2)
N_CORES = 8
F32 = mybir.dt.float32
BF16 = mybir.dt.bfloat16


def build(seq_len=S, debug=False):
    nc = bacc.Bacc(None, target_bir_lowering=False, debug=debug)
    AF = mybir.ActivationFunctionType

    TC = min(64, seq_len)          # timesteps per chunk
    NCH = seq_len // TC            # chunks
    TOK = TC * B                   # tokens per chunk (<= 512)
    QN = TOK // P                  # 128-token subtiles per chunk
    TL = P // B                    # t_locals per subtile (16)

    x_d = nc.dram_tensor("x", [B, seq_len, IN], F32, kind="ExternalInput")
    wf_d = nc.dram_tensor("Wf", [H + IN, H], F32, kind="ExternalInput")
    bf_d = nc.dram_tensor("bf", [H], F32, kind="ExternalInput")
    ws_d = nc.dram_tensor("Ws", [H + IN, H], F32, kind="ExternalInput")
    bs_d = nc.dram_tensor("bs", [H], F32, kind="ExternalInput")
    wo_d = nc.dram_tensor("Wo", [H, O], F32, kind="ExternalInput")
    bo_d = nc.dram_tensor("bo", [O], F32, kind="ExternalInput")
    out_d = nc.dram_tensor("out", [B, seq_len, O], F32, kind="ExternalOutput")
    hfin_d = nc.dram_tensor("hfin", [B, H], F32, kind="ExternalOutput")

    with tile.TileContext(nc) as tc, ExitStack() as ctx:
        dma = nc.sync.dma_start
        cpool = ctx.enter_context(tc.tile_pool(name="const", bufs=1))

        # --- weights/constants in SBUF (bf16 compute copies) ---
        wfh = cpool.tile([P, HC, H], BF16)
        wsh = cpool.tile([P, HC, H], BF16)
        wfx = cpool.tile([P, ICC, H], BF16)
        wsx = cpool.tile([P, ICC, H], BF16)
        wo_sb = cpool.tile([P, HC, O], BF16)
        bft = cpool.tile([P, HC], F32)
        bst = cpool.tile([P, HC], F32)
        ident = cpool.tile([P, P], BF16)
        h0 = cpool.tile([P, HC, B], BF16)
        ones = cpool.tile([1, P], F32)
        bo_row = cpool.tile([1, O], F32)
        bo_bc = cpool.tile([P, O], F32)
        hfin_sb = cpool.tile([B, H], F32)
        stg = cpool.tile([P, HC, H], F32)   # fp32 staging for weight casts

        def load_cast(dst, src_ap, nch):
            dma(stg[:, :nch, :dst.shape[2]], src_ap)
            nc.vector.tensor_copy(dst[:], stg[:, :nch, :dst.shape[2]])

        load_cast(wfh, wf_d[0:H, :].rearrange("(kc p) j -> p kc j", p=P), HC)
        load_cast(wsh, ws_d[0:H, :].rearrange("(kc p) j -> p kc j", p=P), HC)
        load_cast(wfx, wf_d[H:H + IN, :].rearrange("(kc p) j -> p kc j", p=P), ICC)
        load_cast(wsx, ws_d[H:H + IN, :].rearrange("(kc p) j -> p kc j", p=P), ICC)
        load_cast(wo_sb, wo_d.rearrange("(kc p) o -> p kc o", p=P), HC)
        dma(bft[:], bf_d.rearrange("(jc p) -> p jc", p=P))
        dma(bst[:], bs_d.rearrange("(jc p) -> p jc", p=P))
        dma(bo_row[:], bo_d[None, :])
        masks.make_identity(nc, ident[:])
        nc.vector.memset(ones[:], 1.0)
        nc.vector.memset(h0[:], 0.0)

        # psum pools (8 banks: 2+2+1+1+2)
        tp_pool = ctx.enter_context(tc.tile_pool(name="tp", bufs=2, space="PSUM"))
        p1_pool = ctx.enter_context(tc.tile_pool(name="p1", bufs=2, space="PSUM"))
        pf_pool = ctx.enter_context(tc.tile_pool(name="pf", bufs=1, space="PSUM"))
        ps_pool = ctx.enter_context(tc.tile_pool(name="ps", bufs=1, space="PSUM"))
        po_pool = ctx.enter_context(tc.tile_pool(name="po", bufs=2, space="PSUM"))

        # bo broadcast across partitions via ones-matmul
        pbo = po_pool.tile([P, O], F32, tag="po")
        nc.tensor.matmul(pbo[:], ones[:], bo_row[:], start=True, stop=True)
        nc.vector.tensor_copy(bo_bc[:], pbo[:])

        # sbuf pools
        xin_pool = ctx.enter_context(tc.tile_pool(name="xin", bufs=2 * QN))
        xt_pool = ctx.enter_context(tc.tile_pool(name="xt", bufs=2))
        xf_pool = ctx.enter_context(tc.tile_pool(name="xf", bufs=2))
        xs_pool = ctx.enter_context(tc.tile_pool(name="xs", bufs=2))
        ht_pool = ctx.enter_context(tc.tile_pool(name="ht", bufs=2))
        sc_pool = ctx.enter_context(tc.tile_pool(name="sc", bufs=3))
        ob_pool = ctx.enter_context(tc.tile_pool(name="ob", bufs=3))

        def phase1(c):
            """Input projections for chunk c -> (XfT, XsT) [P, HC, TOK] fp32."""
            xins = []
            for q in range(QN):
                xi = xin_pool.tile([P, IN], F32, tag="xin")
                xib = xin_pool.tile([P, IN], BF16, tag="xib")
                t0 = c * TC + q * TL
                t1 = c * TC + (q + 1) * TL
                for b in range(B):
                    dma(xi[b * TL:(b + 1) * TL, :], x_d[b, t0:t1, :])
                nc.vector.tensor_copy(xib[:], xi[:])
                xins.append(xib)
            xtt = xt_pool.tile([P, ICC, TOK], BF16, tag="xt")
            for q in range(QN):
                for kc in range(ICC):
                    tpt = tp_pool.tile([P, P], BF16, tag="tp")
                    nc.tensor.transpose(tpt[:], xins[q][:, kc * P:(kc + 1) * P], ident[:])
                    nc.scalar.copy(xtt[:, kc, q * P:(q + 1) * P], tpt[:])
            xft = xf_pool.tile([P, HC, TOK], F32, tag="xf")
            xst = xs_pool.tile([P, HC, TOK], F32, tag="xs")
            for w_sb, b_t, dst in ((wfx, bft, xft), (wsx, bst, xst)):
                for jc in range(HC):
                    p1t = p1_pool.tile([P, TOK], F32, tag="p1")
                    for kc in range(ICC):
                        nc.tensor.matmul(
                            p1t[:], w_sb[:, kc, jc * P:(jc + 1) * P], xtt[:, kc, :],
                            start=(kc == 0), stop=(kc == ICC - 1))
                    nc.scalar.activation(dst[:, jc, :], p1t[:], AF.Identity,
                                         bias=b_t[:, jc:jc + 1], scale=1.0)
            return xft, xst

        def scan(c, xft, xst, hprev):
            """TC recurrent steps for chunk c; returns HT tile (bf16) + new hprev."""
            htt = ht_pool.tile([P, HC, TOK], BF16, tag="ht")
            htt_v = htt[:].rearrange("p h (q b t) -> p h q t b", b=B, t=TL)
            xft_v = xft[:].rearrange("p h (q b t) -> p h q t b", b=B, t=TL)
            xst_v = xst[:].rearrange("p h (q b t) -> p h q t b", b=B, t=TL)
            for t in range(TC):
                q, tl = t // TL, t % TL
                pf_t = pf_pool.tile([P, HC, B], F32, tag="pf")
                for jc in range(HC):
                    for kc in range(HC):
                        nc.tensor.matmul(
                            pf_t[:, jc, :], wfh[:, kc, jc * P:(jc + 1) * P],
                            hprev[:, kc, :], start=(kc == 0), stop=(kc == HC - 1))
                # zf = psum + Xf  (in place, fp32), f = sigmoid(zf) from PSUM
                nc.vector.tensor_add(pf_t[:], pf_t[:], xft_v[:, :, q, tl, :])
                f = sc_pool.tile([P, HC, B], F32, tag="f")
                nc.scalar.activation(f[:], pf_t[:], AF.Sigmoid)
                g = sc_pool.tile([P, HC, B], BF16, tag="g")
                nc.vector.tensor_mul(g[:], f[:], hprev[:])
                # t1 = h - g = (1-f)*h, off the critical path (runs under s-MMs)
                t1 = sc_pool.tile([P, HC, B], F32, tag="t1")
                nc.vector.tensor_sub(t1[:], hprev[:], g[:])
                ps_t = ps_pool.tile([P, HC, B], F32, tag="ps")
                for jc in range(HC):
                    for kc in range(HC):
                        nc.tensor.matmul(
                            ps_t[:, jc, :], wsh[:, kc, jc * P:(jc + 1) * P],
                            g[:, kc, :], start=(kc == 0), stop=(kc == HC - 1))
                nc.vector.tensor_add(ps_t[:], ps_t[:], xst_v[:, :, q, tl, :])
                s = sc_pool.tile([P, HC, B], F32, tag="s")
                nc.scalar.activation(s[:], ps_t[:], AF.Tanh)
                m = sc_pool.tile([P, HC, B], F32, tag="m")
                nc.vector.tensor_mul(m[:], f[:], s[:])
                hnew = htt_v[:, :, q, tl, :]
                nc.vector.tensor_add(hnew, t1[:], m[:])
                hprev = hnew
            return htt, hprev

        def phase3(c, htt):
            """Output projection + relu for chunk c, DMA to out."""
            for q in range(QN):
                pot = po_pool.tile([P, O], F32, tag="po")
                for kc in range(HC):
                    nc.tensor.matmul(pot[:], htt[:, kc, q * P:(q + 1) * P],
                                     wo_sb[:, kc, :], start=(kc == 0),
                                     stop=(kc == HC - 1))
                ob = ob_pool.tile([P, O], F32, tag="ob")
                nc.vector.tensor_add(ob[:], pot[:], bo_bc[:])
                ob2 = ob_pool.tile([P, O], F32, tag="ob2")
                nc.vector.tensor_scalar_max(ob2[:], ob[:], 0.0)
                t0 = c * TC + q * TL
                t1 = c * TC + (q + 1) * TL
                for b in range(B):
                    dma(out_d[b, t0:t1, :], ob2[b * TL:(b + 1) * TL, :])

        hprev = h0[:, :, :]
        xft, xst = phase1(0)
        for c in range(NCH):
            htt, hprev = scan(c, xft, xst, hprev)
            phase3(c, htt)
            if c + 1 < NCH:
                xft, xst = phase1(c + 1)

        # final state: transpose h_last [P, HC, B] (bf16) -> [B, H] fp32
        for kc in range(HC):
            tpt = tp_pool.tile([P, P], BF16, tag="tp")
            nc.tensor.transpose(tpt[:B, :], hprev[:, kc, :], ident[:])
            nc.vector.tensor_copy(hfin_sb[:, kc * P:(kc + 1) * P], tpt[:B, :])
        dma(hfin_d[:, :], hfin_sb[:])

    nc.compile()
    return nc


_NC_CACHE = {}


def _get_nc():
    if "nc" not in _NC_CACHE:
        _NC_CACHE["nc"] = build()
    return _NC_CACHE["nc"]


def kernel(inputs, Wf, bf, Ws, bs, Wo, bo):
    from concourse.bass_utils import run_bass_kernel_spmd

    inputs = np.ascontiguousarray(np.asarray(inputs, dtype=np.float32))
    Wf = np.ascontiguousarray(np.asarray(Wf, dtype=np.float32))
    bf = np.ascontiguousarray(np.asarray(bf, dtype=np.float32))
    Ws = np.ascontiguousarray(np.asarray(Ws, dtype=np.float32))
    bs = np.ascontiguousarray(np.asarray(bs, dtype=np.float32))
    Wo = np.ascontiguousarray(np.asarray(Wo, dtype=np.float32))
    bo = np.ascontiguousarray(np.asarray(bo, dtype=np.float32))

    nc = _get_nc()
    batch = inputs.shape[0]
    bpc = batch // N_CORES
    in_maps = []
    for i in range(N_CORES):
        in_maps.append({
            "x": inputs[i * bpc:(i + 1) * bpc],
            "Wf": Wf, "bf": bf, "Ws": Ws, "bs": bs, "Wo": Wo, "bo": bo,
        })
    res = run_bass_kernel_spmd(
        nc, in_maps, core_ids=list(range(N_CORES)),
        trace=bool(int(os.environ.get("KERNEL_TRACE", "0"))),
    )
    outs = np.concatenate([r["out"] for r in res.results], axis=0)
    hfin = np.concatenate([r["hfin"] for r in res.results], axis=0)
    _NC_CACHE["last_results"] = res
    return outs, hfin


# revision 23
# speedup vs baseline: 5.1539x; 1.1825x over previous
"""MGU (minimal gated unit) RNN kernel for Trainium2, 8-core data-parallel.

Problem (hardcoded):
    inputs [64, 512, 256] f32, Wf/Ws [768, 512], bf/bs [512], Wo [512, 256], bo [256]
    h_t = (1-f)*h + f*tanh(([f*h, x] @ Ws) + bs),  f = sigmoid(([h, x] @ Wf) + bf)
    outputs = relu(all_hidden @ Wo + bo);  returns (outputs, final_state)

Sharding: batch 64 -> 8 cores x 8. Weights replicated. Scan local per core.

Layout: "transposed-land" — hidden on partitions, (time, batch) on the free dim.
    hidden index h = jc*128 + p; within a 64-step chunk, token col = q*128 + b*16 + t
Scan matmuls are weights-stationary bf16 (fp32 runs as a 4-cycle/row double
pass on this PE; bf16 LDW+MM pairs measure ~42ns vs fp32 ~435ns):
    matmul(psum[:, jc, :], lhsT=Wh_bf[:, kc, jc*128:+128], rhs=h_bf[:, kc, :])
Gate math runs fp32 from PSUM; the hidden state is stored bf16 (matmul rhs +
output-projection stationary). Input/output projections are wide-N bf16
matmuls per 512-token chunk, fully fused (no DRAM scratch).
"""

import os
import sys
import types
from contextlib import ExitStack

import numpy as np

# The agent image's `antenv` package lacks `axon_hooks`, which degrades NTFF
# profiling (trace=True) to a hard ImportError in bass_utils. Pre-seed a
# minimal implementation and register the ctypes-driven hook.
if "antenv.axon_hooks" not in sys.modules:
    _m = types.ModuleType("antenv.axon_hooks")
    _m._hook = None

    def _set_hook(h, _m=_m):
        _m._hook = h

    def _get_hook(_m=_m):
        return _m._hook

    _m.set_axon_ntff_profile_hook = _set_hook
    _m.get_axon_ntff_profile_hook = _get_hook
    sys.modules["antenv.axon_hooks"] = _m
    try:
        from trn_agent_boot.trn_boot import _ntff_profile_via_ctypes

        _h = _ntff_profile_via_ctypes("/opt/axon/libaxon_pjrt.so")
        if _h is not None:
            _set_hook(_h)
    except Exception:
        pass

import concourse.bacc as bacc
import concourse.mybir as mybir
from concourse import masks, tile

P = 128
B = 8          # batch per core
S = 512        # sequence length
IN = 256       # input units
H = 512        # hidden units
O = 256        # output units
HC = H // P    # hidden chunks (4)
ICC = IN // P  # input chunks (2)
N_CORES = 8
F32 = mybir.dt.float32
BF16 = mybir.dt.bfloat16


def build(seq_len=S, debug=False):
    nc = bacc.Bacc(None, target_bir_lowering=False, debug=debug)
    AF = mybir.ActivationFunctionType

    TC = min(64, seq_len)          # timesteps per chunk
    NCH = seq_len // TC            # chunks
    TOK = TC * B                   # tokens per chunk (<= 512)
    QN = TOK // P                  # 128-token subtiles per chunk
    TL = P // B                    # t_locals per subtile (16)

    x_d = nc.dram_tensor("x", [B, seq_len, IN], F32, kind="ExternalInput")
    wf_d = nc.dram_tensor("Wf", [H + IN, H], F32, kind="ExternalInput")
    bf_d = nc.dram_tensor("bf", [H], F32, kind="ExternalInput")
    ws_d = nc.dram_tensor("Ws", [H + IN, H], F32, kind="ExternalInput")
    bs_d = nc.dram_tensor("bs", [H], F32, kind="ExternalInput")
    wo_d = nc.dram_tensor("Wo", [H, O], F32, kind="ExternalInput")
    bo_d = nc.dram_tensor("bo", [O], F32, kind="ExternalInput")
    out_d = nc.dram_tensor("out", [B, seq_len, O], F32, kind="ExternalOutput")
    hfin_d = nc.dram_tensor("hfin", [B, H], F32, kind="ExternalOutput")

    with tile.TileContext(nc) as tc, ExitStack() as ctx:
        dma = nc.sync.dma_start
        cpool = ctx.enter_context(tc.tile_pool(name="const", bufs=1))

        # --- weights/constants in SBUF (bf16 compute copies) ---
        wfh = cpool.tile([P, HC, H], BF16)
        wsh = cpool.tile([P, HC, H], BF16)
        wfx = cpool.tile([P, ICC, H], BF16)
        wsx = cpool.tile([P, ICC, H], BF16)
        wo_sb = cpool.tile([P, HC, O], BF16)
        bft = cpool.tile([P, HC], F32)
        bst = cpool.tile([P, HC], F32)
        ident = cpool.tile([P, P], BF16)
        h0 = cpool.tile([P, HC, B], BF16)
        ones = cpool.tile([1, P], F32)
        bo_row = cpool.tile([1, O], F32)
        bo_bc = cpool.tile([P, O], F32)
        hfin_sb = cpool.tile([B, H], F32)
        stg = cpool.tile([P, HC, H], F32)   # fp32 staging for weight casts

        def load_cast(dst, src_ap, nch):
            dma(stg[:, :nch, :dst.shape[2]], src_ap)
            nc.vector.tensor_copy(dst[:], stg[:, :nch, :dst.shape[2]])

        load_cast(wfh, wf_d[0:H, :].rearrange("(kc p) j -> p kc j", p=P), HC)
        load_cast(wsh, ws_d[0:H, :].rearrange("(kc p) j -> p kc j", p=P), HC)
        load_cast(wfx, wf_d[H:H + IN, :].rearrange("(kc p) j -> p kc j", p=P), ICC)
        load_cast(wsx, ws_d[H:H + IN, :].rearrange("(kc p) j -> p kc j", p=P), ICC)
        load_cast(wo_sb, wo_d.rearrange("(kc p) o -> p kc o", p=P), HC)
        dma(bft[:], bf_d.rearrange("(jc p) -> p jc", p=P))
        dma(bst[:], bs_d.rearrange("(jc p) -> p jc", p=P))
        dma(bo_row[:], bo_d[None, :])
        masks.make_identity(nc, ident[:])
        nc.vector.memset(ones[:], 1.0)
        nc.vector.memset(h0[:], 0.0)

        # psum pools (8 banks: 2+2+1+1+2)
        tp_pool = ctx.enter_context(tc.tile_pool(name="tp", bufs=2, space="PSUM"))
        p1_pool = ctx.enter_context(tc.tile_pool(name="p1", bufs=2, space="PSUM"))
        pf_pool = ctx.enter_context(tc.tile_pool(name="pf", bufs=1, space="PSUM"))
        ps_pool = ctx.enter_context(tc.tile_pool(name="ps", bufs=1, space="PSUM"))
        po_pool = ctx.enter_context(tc.tile_pool(name="po", bufs=2, space="PSUM"))

        # bo broadcast across partitions via ones-matmul
        pbo = po_pool.tile([P, O], F32, tag="po")
        nc.tensor.matmul(pbo[:], ones[:], bo_row[:], start=True, stop=True)
        nc.vector.tensor_copy(bo_bc[:], pbo[:])

        # sbuf pools
        xin_pool = ctx.enter_context(tc.tile_pool(name="xin", bufs=2 * QN))
        xt_pool = ctx.enter_context(tc.tile_pool(name="xt", bufs=2))
        xf_pool = ctx.enter_context(tc.tile_pool(name="xf", bufs=2))
        xs_pool = ctx.enter_context(tc.tile_pool(name="xs", bufs=2))
        ht_pool = ctx.enter_context(tc.tile_pool(name="ht", bufs=2))
        sc_pool = ctx.enter_context(tc.tile_pool(name="sc", bufs=3))
        ob_pool = ctx.enter_context(tc.tile_pool(name="ob", bufs=3))

        def phase1(c):
            """Input projections for chunk c -> (XfT, XsT) [P, HC, TOK] fp32."""
            xins = []
            for q in range(QN):
                xi = xin_pool.tile([P, IN], F32, tag="xin")
                xib = xin_pool.tile([P, IN], BF16, tag="xib")
                t0 = c * TC + q * TL
                t1 = c * TC + (q + 1) * TL
                for b in range(B):
                    dma(xi[b * TL:(b + 1) * TL, :], x_d[b, t0:t1, :])
                nc.vector.tensor_copy(xib[:], xi[:])
                xins.append(xib)
            xtt = xt_pool.tile([P, ICC, TOK], BF16, tag="xt")
            for q in range(QN):
                for kc in range(ICC):
                    tpt = tp_pool.tile([P, P], BF16, tag="tp")
                    nc.tensor.transpose(tpt[:], xins[q][:, kc * P:(kc + 1) * P], ident[:])
                    nc.scalar.copy(xtt[:, kc, q * P:(q + 1) * P], tpt[:])
            xft = xf_pool.tile([P, HC, TOK], F32, tag="xf")
            xst = xs_pool.tile([P, HC, TOK], F32, tag="xs")
            for w_sb, b_t, dst in ((wfx, bft, xft), (wsx, bst, xst)):
                for jc in range(HC):
                    p1t = p1_pool.tile([P, TOK], F32, tag="p1")
                    for kc in range(ICC):
                        nc.tensor.matmul(
                            p1t[:], w_sb[:, kc, jc * P:(jc + 1) * P], xtt[:, kc, :],
                            start=(kc == 0), stop=(kc == ICC - 1))
                    nc.scalar.activation(dst[:, jc, :], p1t[:], AF.Identity,
                                         bias=b_t[:, jc:jc + 1], scale=1.0)
            return xft, xst

        def scan(c, xft, xst, hprev, pre):
            """TC recurrent steps for chunk c; returns HT tile (bf16) + state.

            `pre` is (pf_t, ps_t) PSUM tiles preloaded with Xf/Xs for this
            chunk's step 0 (or None). Steps with a preloaded psum run their
            matmuls with start=False, accumulating onto the preloaded input
            contribution (has_written bits persist from the prior step's
            matmuls over the same bank region); sigmoid/tanh then read PSUM
            directly. Non-preloaded steps use start=True + a DVE add.
            """
            htt = ht_pool.tile([P, HC, TOK], BF16, tag="ht")
            htt_v = htt[:].rearrange("p h (q b t) -> p h q t b", b=B, t=TL)
            xft_v = xft[:].rearrange("p h (q b t) -> p h q t b", b=B, t=TL)
            xst_v = xst[:].rearrange("p h (q b t) -> p h q t b", b=B, t=TL)

            def slot(t):
                return t // TL, t % TL

            for t in range(TC):
                q, tl = slot(t)
                if c == 0 and t == 0:
                    # h0 = 0: f = sigmoid(Xf), s = tanh(Xs), h1 = f*s
                    f = sc_pool.tile([P, HC, B], F32, tag="f")
                    nc.scalar.activation(f[:], xft_v[:, :, q, tl, :], AF.Sigmoid)
                    s = sc_pool.tile([P, HC, B], F32, tag="s")
                    nc.scalar.activation(s[:], xst_v[:, :, q, tl, :], AF.Tanh)
                    hnew = htt_v[:, :, q, tl, :]
                    nc.vector.tensor_mul(hnew, f[:], s[:])
                    hprev = hnew
                    continue

                preloaded = pre is not None
                if preloaded:
                    pf_t, ps_t = pre
                else:
                    pf_t = pf_pool.tile([P, HC, B], F32, tag="pf")
                    ps_t = ps_pool.tile([P, HC, B], F32, tag="ps")

                for jc in range(HC):
                    for kc in range(HC):
                        nc.tensor.matmul(
                            pf_t[:, jc, :], wfh[:, kc, jc * P:(jc + 1) * P],
                            hprev[:, kc, :],
                            start=(kc == 0 and jc == 0 and not preloaded),
                            stop=(kc == HC - 1),
                            skip_group_check=(preloaded or jc > 0))
                if not preloaded:
                    nc.vector.tensor_add(pf_t[:], pf_t[:], xft_v[:, :, q, tl, :])
                f = sc_pool.tile([P, HC, B], F32, tag="f")
                nc.scalar.activation(f[:], pf_t[:], AF.Sigmoid)
                # preload next step's Xf into pf (ScalarE; same-engine order
                # guarantees it lands after this step's sigmoid read)
                npre = None
                if t + 1 < TC:
                    nq, ntl = slot(t + 1)
                    npf = pf_pool.tile([P, HC, B], F32, tag="pf")
                    nc.scalar.copy(npf[:], xft_v[:, :, nq, ntl, :])
                g = sc_pool.tile([P, HC, B], BF16, tag="g")
                nc.vector.tensor_mul(g[:], f[:], hprev[:])
                # t1 = h - g = (1-f)*h, off the critical path (runs under s-MMs)
                t1 = sc_pool.tile([P, HC, B], F32, tag="t1")
                nc.vector.tensor_sub(t1[:], hprev[:], g[:])
                for jc in range(HC):
                    for kc in range(HC):
                        nc.tensor.matmul(
                            ps_t[:, jc, :], wsh[:, kc, jc * P:(jc + 1) * P],
                            g[:, kc, :],
                            start=(kc == 0 and jc == 0 and not preloaded),
                            stop=(kc == HC - 1),
                            skip_group_check=(preloaded or jc > 0))
                if not preloaded:
                    nc.vector.tensor_add(ps_t[:], ps_t[:], xst_v[:, :, q, tl, :])
                s = sc_pool.tile([P, HC, B], F32, tag="s")
                nc.scalar.activation(s[:], ps_t[:], AF.Tanh)
                if t + 1 < TC:
                    nq, ntl = slot(t + 1)
                    nps = ps_pool.tile([P, HC, B], F32, tag="ps")
                    nc.scalar.copy(nps[:], xst_v[:, :, nq, ntl, :])
                    npre = (npf, nps)
                pre = npre
                m = sc_pool.tile([P, HC, B], F32, tag="m")
                nc.vector.tensor_mul(m[:], f[:], s[:])
                hnew = htt_v[:, :, q, tl, :]
                nc.vector.tensor_add(hnew, t1[:], m[:])
                hprev = hnew
            return htt, hprev

        def phase3(c, htt):
            """Output projection + relu for chunk c, DMA to out."""
            for q in range(QN):
                pot = po_pool.tile([P, O], F32, tag="po")
                for kc in range(HC):
                    nc.tensor.matmul(pot[:], htt[:, kc, q * P:(q + 1) * P],
                                     wo_sb[:, kc, :], start=(kc == 0),
                                     stop=(kc == HC - 1))
                ob = ob_pool.tile([P, O], F32, tag="ob")
                nc.vector.tensor_add(ob[:], pot[:], bo_bc[:])
                ob2 = ob_pool.tile([P, O], F32, tag="ob2")
                nc.vector.tensor_scalar_max(ob2[:], ob[:], 0.0)
                t0 = c * TC + q * TL
                t1 = c * TC + (q + 1) * TL
                for b in range(B):
                    dma(out_d[b, t0:t1, :], ob2[b * TL:(b + 1) * TL, :])

        hprev = h0[:, :, :]
        xft, xst = phase1(0)
        for c in range(NCH):
            htt, hprev = scan(c, xft, xst, hprev, None)
            phase3(c, htt)
            if c + 1 < NCH:
                xft, xst = phase1(c + 1)

        # final state: transpose h_last [P, HC, B] (bf16) -> [B, H] fp32
        for kc in range(HC):
            tpt = tp_pool.tile([P, P], BF16, tag="tp")
            nc.tensor.transpose(tpt[:B, :], hprev[:, kc, :], ident[:])
            nc.vector.tensor_copy(hfin_sb[:, kc * P:(kc + 1) * P], tpt[:B, :])
        dma(hfin_d[:, :], hfin_sb[:])

    nc.compile()
    return nc


_NC_CACHE = {}


def _get_nc():
    if "nc" not in _NC_CACHE:
        _NC_CACHE["nc"] = build()
    return _NC_CACHE["nc"]


def kernel(inputs, Wf, bf, Ws, bs, Wo, bo):
    from concourse.bass_utils import run_bass_kernel_spmd

    inputs = np.ascontiguousarray(np.asarray(inputs, dtype=np.float32))
    Wf = np.ascontiguousarray(np.asarray(Wf, dtype=np.float32))
    bf = np.ascontiguousarray(np.asarray(bf, dtype=np.float32))
    Ws = np.ascontiguousarray(np.asarray(Ws, dtype=np.float32))
    bs = np.ascontiguousarray(np.asarray(bs, dtype=np.float32))
    Wo = np.ascontiguousarray(np.asarray(Wo, dtype=np.float32))
    bo = np.ascontiguousarray(np.asarray(bo, dtype=np.float32))

    nc = _get_nc()
    batch = inputs.shape[0]
    bpc = batch // N_CORES
    in_maps = []
    for i in range(N_CORES):
        in_maps.append({
            "x": inputs[i * bpc:(i + 1) * bpc],
            "Wf": Wf, "bf": bf, "Ws": Ws, "bs": bs, "Wo": Wo, "bo": bo,
        })
    res = run_bass_kernel_spmd(
        nc, in_maps, core_ids=list(range(N_CORES)),
        trace=bool(int(os.environ.get("KERNEL_TRACE", "0"))),
    )
    outs = np.concatenate([r["out"] for r in res.results], axis=0)
    hfin = np.concatenate([r["hfin"] for r in res.results], axis=0)
    _NC_CACHE["last_results"] = res
    return outs, hfin


# revision 26
# speedup vs baseline: 5.2665x; 1.0218x over previous
"""MGU (minimal gated unit) RNN kernel for Trainium2, 8-core data-parallel.

Problem (hardcoded):
    inputs [64, 512, 256] f32, Wf/Ws [768, 512], bf/bs [512], Wo [512, 256], bo [256]
    h_t = (1-f)*h + f*tanh(([f*h, x] @ Ws) + bs),  f = sigmoid(([h, x] @ Wf) + bf)
    outputs = relu(all_hidden @ Wo + bo);  returns (outputs, final_state)

Sharding: batch 64 -> 8 cores x 8. Weights replicated. Scan local per core.

Layout: "transposed-land" — hidden on partitions, (time, batch) on the free dim.
    hidden index h = jc*128 + p; within a 64-step chunk, token col = q*128 + b*16 + t
Scan matmuls are weights-stationary bf16 (fp32 runs as a 4-cycle/row double
pass on this PE; bf16 LDW+MM pairs measure ~42ns vs fp32 ~435ns):
    matmul(psum[:, jc, :], lhsT=Wh_bf[:, kc, jc*128:+128], rhs=h_bf[:, kc, :])
Gate math runs fp32 from PSUM; the hidden state is stored bf16 (matmul rhs +
output-projection stationary). Input/output projections are wide-N bf16
matmuls per 512-token chunk, fully fused (no DRAM scratch).
"""

import os
import sys
import types
from contextlib import ExitStack

import numpy as np

# The agent image's `antenv` package lacks `axon_hooks`, which degrades NTFF
# profiling (trace=True) to a hard ImportError in bass_utils. Pre-seed a
# minimal implementation and register the ctypes-driven hook.
if "antenv.axon_hooks" not in sys.modules:
    _m = types.ModuleType("antenv.axon_hooks")
    _m._hook = None

    def _set_hook(h, _m=_m):
        _m._hook = h

    def _get_hook(_m=_m):
        return _m._hook

    _m.set_axon_ntff_profile_hook = _set_hook
    _m.get_axon_ntff_profile_hook = _get_hook
    sys.modules["antenv.axon_hooks"] = _m
    try:
        from trn_agent_boot.trn_boot import _ntff_profile_via_ctypes

        _h = _ntff_profile_via_ctypes("/opt/axon/libaxon_pjrt.so")
        if _h is not None:
            _set_hook(_h)
    except Exception:
        pass

import concourse.bacc as bacc
import concourse.mybir as mybir
from concourse import masks, tile

P = 128
B = 8          # batch per core
S = 512        # sequence length
IN = 256       # input units
H = 512        # hidden units
O = 256        # output units
HC = H // P    # hidden chunks (4)
ICC = IN // P  # input chunks (2)
N_CORES = 8
F32 = mybir.dt.float32
BF16 = mybir.dt.bfloat16


def build(seq_len=S, debug=False):
    nc = bacc.Bacc(None, target_bir_lowering=False, debug=debug)
    AF = mybir.ActivationFunctionType

    TC = min(64, seq_len)          # timesteps per chunk
    NCH = seq_len // TC            # chunks
    TOK = TC * B                   # tokens per chunk (<= 512)
    QN = TOK // P                  # 128-token subtiles per chunk
    TL = P // B                    # t_locals per subtile (16)

    x_d = nc.dram_tensor("x", [B, seq_len, IN], F32, kind="ExternalInput")
    wf_d = nc.dram_tensor("Wf", [H + IN, H], F32, kind="ExternalInput")
    bf_d = nc.dram_tensor("bf", [H], F32, kind="ExternalInput")
    ws_d = nc.dram_tensor("Ws", [H + IN, H], F32, kind="ExternalInput")
    bs_d = nc.dram_tensor("bs", [H], F32, kind="ExternalInput")
    wo_d = nc.dram_tensor("Wo", [H, O], F32, kind="ExternalInput")
    bo_d = nc.dram_tensor("bo", [O], F32, kind="ExternalInput")
    out_d = nc.dram_tensor("out", [B, seq_len, O], F32, kind="ExternalOutput")
    hfin_d = nc.dram_tensor("hfin", [B, H], F32, kind="ExternalOutput")

    with tile.TileContext(nc) as tc, ExitStack() as ctx:
        dma = nc.sync.dma_start
        cpool = ctx.enter_context(tc.tile_pool(name="const", bufs=1))

        # --- weights/constants in SBUF (bf16 compute copies) ---
        wfh = cpool.tile([P, HC, H], BF16)
        wsh = cpool.tile([P, HC, H], BF16)
        wfx = cpool.tile([P, ICC, H], BF16)
        wsx = cpool.tile([P, ICC, H], BF16)
        wo_sb = cpool.tile([P, HC, O], BF16)
        bft = cpool.tile([P, HC], F32)
        bst = cpool.tile([P, HC], F32)
        ident = cpool.tile([P, P], BF16)
        h0 = cpool.tile([P, HC, B], BF16)
        ones = cpool.tile([1, P], F32)
        bo_row = cpool.tile([1, O], F32)
        bo_bc = cpool.tile([P, O], F32)
        hfin_sb = cpool.tile([B, H], F32)
        stg = cpool.tile([P, HC, H], F32)   # fp32 staging for weight casts

        def load_cast(dst, src_ap, nch):
            dma(stg[:, :nch, :dst.shape[2]], src_ap)
            nc.vector.tensor_copy(dst[:], stg[:, :nch, :dst.shape[2]])

        load_cast(wfh, wf_d[0:H, :].rearrange("(kc p) j -> p kc j", p=P), HC)
        load_cast(wsh, ws_d[0:H, :].rearrange("(kc p) j -> p kc j", p=P), HC)
        load_cast(wfx, wf_d[H:H + IN, :].rearrange("(kc p) j -> p kc j", p=P), ICC)
        load_cast(wsx, ws_d[H:H + IN, :].rearrange("(kc p) j -> p kc j", p=P), ICC)
        load_cast(wo_sb, wo_d.rearrange("(kc p) o -> p kc o", p=P), HC)
        dma(bft[:], bf_d.rearrange("(jc p) -> p jc", p=P))
        dma(bst[:], bs_d.rearrange("(jc p) -> p jc", p=P))
        dma(bo_row[:], bo_d[None, :])
        masks.make_identity(nc, ident[:])
        nc.vector.memset(ones[:], 1.0)
        nc.vector.memset(h0[:], 0.0)

        # psum pools (8 banks: 2+2+1+1+2)
        tp_pool = ctx.enter_context(tc.tile_pool(name="tp", bufs=2, space="PSUM"))
        p1_pool = ctx.enter_context(tc.tile_pool(name="p1", bufs=2, space="PSUM"))
        pf_pool = ctx.enter_context(tc.tile_pool(name="pf", bufs=1, space="PSUM"))
        ps_pool = ctx.enter_context(tc.tile_pool(name="ps", bufs=1, space="PSUM"))
        po_pool = ctx.enter_context(tc.tile_pool(name="po", bufs=2, space="PSUM"))

        # bo broadcast across partitions via ones-matmul
        pbo = po_pool.tile([P, O], F32, tag="po")
        nc.tensor.matmul(pbo[:], ones[:], bo_row[:], start=True, stop=True)
        nc.vector.tensor_copy(bo_bc[:], pbo[:])

        # sbuf pools
        xin_pool = ctx.enter_context(tc.tile_pool(name="xin", bufs=2 * QN))
        xt_pool = ctx.enter_context(tc.tile_pool(name="xt", bufs=2))
        xf_pool = ctx.enter_context(tc.tile_pool(name="xf", bufs=2))
        xs_pool = ctx.enter_context(tc.tile_pool(name="xs", bufs=2))
        ht_pool = ctx.enter_context(tc.tile_pool(name="ht", bufs=2))
        sc_pool = ctx.enter_context(tc.tile_pool(name="sc", bufs=3))
        ob_pool = ctx.enter_context(tc.tile_pool(name="ob", bufs=3))

        def phase1(c):
            """Input projections for chunk c -> (XfT, XsT) [P, HC, TOK] fp32."""
            xins = []
            for q in range(QN):
                xi = xin_pool.tile([P, IN], F32, tag="xin")
                xib = xin_pool.tile([P, IN], BF16, tag="xib")
                t0 = c * TC + q * TL
                t1 = c * TC + (q + 1) * TL
                for b in range(B):
                    dma(xi[b * TL:(b + 1) * TL, :], x_d[b, t0:t1, :])
                nc.vector.tensor_copy(xib[:], xi[:])
                xins.append(xib)
            xtt = xt_pool.tile([P, ICC, TOK], BF16, tag="xt")
            for q in range(QN):
                for kc in range(ICC):
                    tpt = tp_pool.tile([P, P], BF16, tag="tp")
                    nc.tensor.transpose(tpt[:], xins[q][:, kc * P:(kc + 1) * P], ident[:])
                    nc.scalar.copy(xtt[:, kc, q * P:(q + 1) * P], tpt[:])
            xft = xf_pool.tile([P, HC, TOK], F32, tag="xf")
            xst = xs_pool.tile([P, HC, TOK], F32, tag="xs")
            for w_sb, b_t, dst in ((wfx, bft, xft), (wsx, bst, xst)):
                for jc in range(HC):
                    p1t = p1_pool.tile([P, TOK], F32, tag="p1")
                    for kc in range(ICC):
                        nc.tensor.matmul(
                            p1t[:], w_sb[:, kc, jc * P:(jc + 1) * P], xtt[:, kc, :],
                            start=(kc == 0), stop=(kc == ICC - 1))
                    nc.scalar.activation(dst[:, jc, :], p1t[:], AF.Identity,
                                         bias=b_t[:, jc:jc + 1], scale=1.0)
            return xft, xst

        def scan(c, xft, xst, hprev, pre):
            """TC recurrent steps for chunk c; returns HT tile (bf16) + state.

            `pre` is (pf_t, ps_t) PSUM tiles preloaded with Xf/Xs for this
            chunk's step 0 (or None). Steps with a preloaded psum run their
            matmuls with start=False, accumulating onto the preloaded input
            contribution (has_written bits persist from the prior step's
            matmuls over the same bank region); sigmoid/tanh then read PSUM
            directly. Non-preloaded steps use start=True + a DVE add.
            """
            htt = ht_pool.tile([P, HC, TOK], BF16, tag="ht")
            htt_v = htt[:].rearrange("p h (q b t) -> p h q t b", b=B, t=TL)
            xft_v = xft[:].rearrange("p h (q b t) -> p h q t b", b=B, t=TL)
            xst_v = xst[:].rearrange("p h (q b t) -> p h q t b", b=B, t=TL)

            def slot(t):
                return t // TL, t % TL

            for t in range(TC):
                q, tl = slot(t)
                if c == 0 and t == 0:
                    # h0 = 0: f = sigmoid(Xf), s = tanh(Xs), h1 = f*s
                    f = sc_pool.tile([P, HC, B], F32, tag="f")
                    nc.scalar.activation(f[:], xft_v[:, :, q, tl, :], AF.Sigmoid)
                    s = sc_pool.tile([P, HC, B], F32, tag="s")
                    nc.scalar.activation(s[:], xst_v[:, :, q, tl, :], AF.Tanh)
                    hnew = htt_v[:, :, q, tl, :]
                    nc.vector.tensor_mul(hnew, f[:], s[:])
                    hprev = hnew
                    continue

                preloaded = pre is not None
                if preloaded:
                    pf_t, ps_t = pre
                else:
                    pf_t = pf_pool.tile([P, HC, B], F32, tag="pf")
                    ps_t = ps_pool.tile([P, HC, B], F32, tag="ps")

                # f-matmuls kc-outer: the kc-group only needs h'[kc] from the
                # previous step, so next-step matmuls overlap this step's tail.
                for kc in range(HC):
                    for jc in range(HC):
                        nc.tensor.matmul(
                            pf_t[:, jc, :], wfh[:, kc, jc * P:(jc + 1) * P],
                            hprev[:, kc, :],
                            start=(kc == 0 and jc == 0 and not preloaded),
                            stop=True,
                            skip_group_check=(preloaded or kc > 0 or jc > 0))
                if not preloaded:
                    nc.vector.tensor_add(pf_t[:], pf_t[:], xft_v[:, :, q, tl, :])
                f = sc_pool.tile([P, HC, B], F32, tag="f")
                nc.scalar.activation(f[:], pf_t[:], AF.Sigmoid)
                # preload next step's Xf into pf (ScalarE; same-engine order
                # guarantees it lands after this step's sigmoid read)
                npre = None
                if t + 1 < TC:
                    nq, ntl = slot(t + 1)
                    npf = pf_pool.tile([P, HC, B], F32, tag="pf")
                    nc.scalar.copy(npf[:], xft_v[:, :, nq, ntl, :])
                HH = HC // 2
                g = sc_pool.tile([P, HC, B], BF16, tag="g")
                t1 = sc_pool.tile([P, HC, B], F32, tag="t1")
                for h_ in range(2):
                    lo, hi = h_ * HH, (h_ + 1) * HH
                    nc.vector.tensor_mul(g[:, lo:hi, :], f[:, lo:hi, :],
                                         hprev[:, lo:hi, :])
                for h_ in range(2):
                    lo, hi = h_ * HH, (h_ + 1) * HH
                    # t1 = h - g = (1-f)*h, off the critical path
                    nc.vector.tensor_sub(t1[:, lo:hi, :], hprev[:, lo:hi, :],
                                         g[:, lo:hi, :])
                # s-matmuls kc-outer: kc-group only needs g[kc]
                for kc in range(HC):
                    for jc in range(HC):
                        nc.tensor.matmul(
                            ps_t[:, jc, :], wsh[:, kc, jc * P:(jc + 1) * P],
                            g[:, kc, :],
                            start=(kc == 0 and jc == 0 and not preloaded),
                            stop=True,
                            skip_group_check=(preloaded or kc > 0 or jc > 0))
                if not preloaded:
                    nc.vector.tensor_add(ps_t[:], ps_t[:], xst_v[:, :, q, tl, :])
                s = sc_pool.tile([P, HC, B], F32, tag="s")
                m = sc_pool.tile([P, HC, B], F32, tag="m")
                hnew = htt_v[:, :, q, tl, :]
                for h_ in range(2):
                    lo, hi = h_ * HH, (h_ + 1) * HH
                    nc.scalar.activation(s[:, lo:hi, :], ps_t[:, lo:hi, :], AF.Tanh)
                if t + 1 < TC:
                    nq, ntl = slot(t + 1)
                    nps = ps_pool.tile([P, HC, B], F32, tag="ps")
                    nc.scalar.copy(nps[:], xst_v[:, :, nq, ntl, :])
                    npre = (npf, nps)
                pre = npre
                for h_ in range(2):
                    lo, hi = h_ * HH, (h_ + 1) * HH
                    nc.vector.tensor_mul(m[:, lo:hi, :], f[:, lo:hi, :],
                                         s[:, lo:hi, :])
                    nc.vector.tensor_add(hnew[:, lo:hi, :], t1[:, lo:hi, :],
                                         m[:, lo:hi, :])
                hprev = hnew
            return htt, hprev

        def phase3(c, htt):
            """Output projection + relu for chunk c, DMA to out."""
            for q in range(QN):
                pot = po_pool.tile([P, O], F32, tag="po")
                for kc in range(HC):
                    nc.tensor.matmul(pot[:], htt[:, kc, q * P:(q + 1) * P],
                                     wo_sb[:, kc, :], start=(kc == 0),
                                     stop=(kc == HC - 1))
                ob = ob_pool.tile([P, O], F32, tag="ob")
                nc.vector.tensor_add(ob[:], pot[:], bo_bc[:])
                ob2 = ob_pool.tile([P, O], F32, tag="ob2")
                nc.vector.tensor_scalar_max(ob2[:], ob[:], 0.0)
                t0 = c * TC + q * TL
                t1 = c * TC + (q + 1) * TL
                for b in range(B):
                    dma(out_d[b, t0:t1, :], ob2[b * TL:(b + 1) * TL, :])

        hprev = h0[:, :, :]
        xft, xst = phase1(0)
        for c in range(NCH):
            htt, hprev = scan(c, xft, xst, hprev, None)
            phase3(c, htt)
            if c + 1 < NCH:
                xft, xst = phase1(c + 1)

        # final state: transpose h_last [P, HC, B] (bf16) -> [B, H] fp32
        for kc in range(HC):
            tpt = tp_pool.tile([P, P], BF16, tag="tp")
            nc.tensor.transpose(tpt[:B, :], hprev[:, kc, :], ident[:])
            nc.vector.tensor_copy(hfin_sb[:, kc * P:(kc + 1) * P], tpt[:B, :])
        dma(hfin_d[:, :], hfin_sb[:])

    nc.compile()
    return nc


_NC_CACHE = {}


def _get_nc():
    if "nc" not in _NC_CACHE:
        _NC_CACHE["nc"] = build()
    return _NC_CACHE["nc"]


def kernel(inputs, Wf, bf, Ws, bs, Wo, bo):
    from concourse.bass_utils import run_bass_kernel_spmd

    inputs = np.ascontiguousarray(np.asarray(inputs, dtype=np.float32))
    Wf = np.ascontiguousarray(np.asarray(Wf, dtype=np.float32))
    bf = np.ascontiguousarray(np.asarray(bf, dtype=np.float32))
    Ws = np.ascontiguousarray(np.asarray(Ws, dtype=np.float32))
    bs = np.ascontiguousarray(np.asarray(bs, dtype=np.float32))
    Wo = np.ascontiguousarray(np.asarray(Wo, dtype=np.float32))
    bo = np.ascontiguousarray(np.asarray(bo, dtype=np.float32))

    nc = _get_nc()
    batch = inputs.shape[0]
    bpc = batch // N_CORES
    in_maps = []
    for i in range(N_CORES):
        in_maps.append({
            "x": inputs[i * bpc:(i + 1) * bpc],
            "Wf": Wf, "bf": bf, "Ws": Ws, "bs": bs, "Wo": Wo, "bo": bo,
        })
    res = run_bass_kernel_spmd(
        nc, in_maps, core_ids=list(range(N_CORES)),
        trace=bool(int(os.environ.get("KERNEL_TRACE", "0"))),
    )
    outs = np.concatenate([r["out"] for r in res.results], axis=0)
    hfin = np.concatenate([r["hfin"] for r in res.results], axis=0)
    _NC_CACHE["last_results"] = res
    return outs, hfin


# revision 29
# speedup vs baseline: 5.8761x; 1.1158x over previous
"""MGU (minimal gated unit) RNN kernel for Trainium2, 8-core data-parallel.

Problem (hardcoded):
    inputs [64, 512, 256] f32, Wf/Ws [768, 512], bf/bs [512], Wo [512, 256], bo [256]
    h_t = (1-f)*h + f*tanh(([f*h, x] @ Ws) + bs),  f = sigmoid(([h, x] @ Wf) + bf)
    outputs = relu(all_hidden @ Wo + bo);  returns (outputs, final_state)

Sharding: batch 64 -> 8 cores x 8. Weights replicated. Scan local per core.

Layout: "transposed-land" — hidden on partitions, (time, batch) on the free dim.
    hidden index h = jc*128 + p; within a 64-step chunk, token col = q*128 + b*16 + t
Scan matmuls are weights-stationary bf16 (fp32 runs as a 4-cycle/row double
pass on this PE; bf16 LDW+MM pairs measure ~42ns vs fp32 ~435ns):
    matmul(psum[:, jc, :], lhsT=Wh_bf[:, kc, jc*128:+128], rhs=h_bf[:, kc, :])
Gate math runs fp32 from PSUM; the hidden state is stored bf16 (matmul rhs +
output-projection stationary). Input/output projections are wide-N bf16
matmuls per 512-token chunk, fully fused (no DRAM scratch).
"""

import os
import sys
import types
from contextlib import ExitStack

import numpy as np

# The agent image's `antenv` package lacks `axon_hooks`, which degrades NTFF
# profiling (trace=True) to a hard ImportError in bass_utils. Pre-seed a
# minimal implementation and register the ctypes-driven hook.
if "antenv.axon_hooks" not in sys.modules:
    _m = types.ModuleType("antenv.axon_hooks")
    _m._hook = None

    def _set_hook(h, _m=_m):
        _m._hook = h

    def _get_hook(_m=_m):
        return _m._hook

    _m.set_axon_ntff_profile_hook = _set_hook
    _m.get_axon_ntff_profile_hook = _get_hook
    sys.modules["antenv.axon_hooks"] = _m
    try:
        from trn_agent_boot.trn_boot import _ntff_profile_via_ctypes

        _h = _ntff_profile_via_ctypes("/opt/axon/libaxon_pjrt.so")
        if _h is not None:
            _set_hook(_h)
    except Exception:
        pass

import concourse.bacc as bacc
import concourse.mybir as mybir
from concourse import masks, tile

P = 128
B = 8          # batch per core
S = 512        # sequence length
IN = 256       # input units
H = 512        # hidden units
O = 256        # output units
HC = H // P    # hidden chunks (4)
ICC = IN // P  # input chunks (2)
N_CORES = 8
F32 = mybir.dt.float32
BF16 = mybir.dt.bfloat16


def build(seq_len=S, debug=False):
    nc = bacc.Bacc(None, target_bir_lowering=False, debug=debug)
    AF = mybir.ActivationFunctionType

    TC = min(64, seq_len)          # timesteps per chunk
    NCH = seq_len // TC            # chunks
    TOK = TC * B                   # tokens per chunk (<= 512)
    QN = TOK // P                  # 128-token subtiles per chunk
    TL = P // B                    # t_locals per subtile (16)

    x_d = nc.dram_tensor("x", [B, seq_len, IN], F32, kind="ExternalInput")
    wf_d = nc.dram_tensor("Wf", [H + IN, H], F32, kind="ExternalInput")
    bf_d = nc.dram_tensor("bf", [H], F32, kind="ExternalInput")
    ws_d = nc.dram_tensor("Ws", [H + IN, H], F32, kind="ExternalInput")
    bs_d = nc.dram_tensor("bs", [H], F32, kind="ExternalInput")
    wo_d = nc.dram_tensor("Wo", [H, O], F32, kind="ExternalInput")
    bo_d = nc.dram_tensor("bo", [O], F32, kind="ExternalInput")
    out_d = nc.dram_tensor("out", [B, seq_len, O], F32, kind="ExternalOutput")
    hfin_d = nc.dram_tensor("hfin", [B, H], F32, kind="ExternalOutput")

    with tile.TileContext(nc) as tc, ExitStack() as ctx:
        dma = nc.sync.dma_start
        cpool = ctx.enter_context(tc.tile_pool(name="const", bufs=1))

        # --- weights/constants in SBUF (bf16 compute copies) ---
        wfh = cpool.tile([P, HC, H], BF16)
        wsh = cpool.tile([P, HC, H], BF16)
        wfx = cpool.tile([P, ICC, H], BF16)
        wsx = cpool.tile([P, ICC, H], BF16)
        wo_sb = cpool.tile([P, HC, O], BF16)
        bft = cpool.tile([P, HC], F32)
        bst = cpool.tile([P, HC], F32)
        ident = cpool.tile([P, P], BF16)
        h0 = cpool.tile([P, HC, B], BF16)
        ones = cpool.tile([1, P], F32)
        bo_row = cpool.tile([1, O], F32)
        bo_bc = cpool.tile([P, O], F32)
        hfin_sb = cpool.tile([B, H], F32)
        stg = cpool.tile([P, HC, H], F32)   # fp32 staging for weight casts

        def load_cast(dst, src_ap, nch):
            dma(stg[:, :nch, :dst.shape[2]], src_ap)
            nc.vector.tensor_copy(dst[:], stg[:, :nch, :dst.shape[2]])

        load_cast(wfh, wf_d[0:H, :].rearrange("(kc p) j -> p kc j", p=P), HC)
        load_cast(wsh, ws_d[0:H, :].rearrange("(kc p) j -> p kc j", p=P), HC)
        load_cast(wfx, wf_d[H:H + IN, :].rearrange("(kc p) j -> p kc j", p=P), ICC)
        load_cast(wsx, ws_d[H:H + IN, :].rearrange("(kc p) j -> p kc j", p=P), ICC)
        load_cast(wo_sb, wo_d.rearrange("(kc p) o -> p kc o", p=P), HC)
        dma(bft[:], bf_d.rearrange("(jc p) -> p jc", p=P))
        dma(bst[:], bs_d.rearrange("(jc p) -> p jc", p=P))
        dma(bo_row[:], bo_d[None, :])
        masks.make_identity(nc, ident[:])
        nc.vector.memset(ones[:], 1.0)
        nc.vector.memset(h0[:], 0.0)

        # psum pools (8 banks: 1+1+1+1+1+1+2). The f/s gate accumulators are
        # split into per-half banks so sigmoid/tanh of the first half can run
        # while the PE still writes the second half's bank (the PSUM bank
        # rule serializes any reader behind all same-bank PE writes).
        tp_pool = ctx.enter_context(tc.tile_pool(name="tp", bufs=1, space="PSUM"))
        p1_pool = ctx.enter_context(tc.tile_pool(name="p1", bufs=1, space="PSUM"))
        pfa_pool = ctx.enter_context(tc.tile_pool(name="pfa", bufs=1, space="PSUM"))
        pfb_pool = ctx.enter_context(tc.tile_pool(name="pfb", bufs=1, space="PSUM"))
        psa_pool = ctx.enter_context(tc.tile_pool(name="psa", bufs=1, space="PSUM"))
        psb_pool = ctx.enter_context(tc.tile_pool(name="psb", bufs=1, space="PSUM"))
        po_pool = ctx.enter_context(tc.tile_pool(name="po", bufs=2, space="PSUM"))

        # bo broadcast across partitions via ones-matmul
        pbo = po_pool.tile([P, O], F32, tag="po")
        nc.tensor.matmul(pbo[:], ones[:], bo_row[:], start=True, stop=True)
        nc.vector.tensor_copy(bo_bc[:], pbo[:])

        # sbuf pools
        xin_pool = ctx.enter_context(tc.tile_pool(name="xin", bufs=2 * QN))
        xt_pool = ctx.enter_context(tc.tile_pool(name="xt", bufs=2))
        xf_pool = ctx.enter_context(tc.tile_pool(name="xf", bufs=2))
        xs_pool = ctx.enter_context(tc.tile_pool(name="xs", bufs=2))
        ht_pool = ctx.enter_context(tc.tile_pool(name="ht", bufs=2))
        sc_pool = ctx.enter_context(tc.tile_pool(name="sc", bufs=3))
        ob_pool = ctx.enter_context(tc.tile_pool(name="ob", bufs=3))

        def phase1(c):
            """Input projections for chunk c -> (XfT, XsT) [P, HC, TOK] fp32."""
            xins = []
            for q in range(QN):
                xi = xin_pool.tile([P, IN], F32, tag="xin")
                xib = xin_pool.tile([P, IN], BF16, tag="xib")
                t0 = c * TC + q * TL
                t1 = c * TC + (q + 1) * TL
                for b in range(B):
                    dma(xi[b * TL:(b + 1) * TL, :], x_d[b, t0:t1, :])
                nc.vector.tensor_copy(xib[:], xi[:])
                xins.append(xib)
            xtt = xt_pool.tile([P, ICC, TOK], BF16, tag="xt")
            for q in range(QN):
                for kc in range(ICC):
                    tpt = tp_pool.tile([P, P], BF16, tag="tp")
                    nc.tensor.transpose(tpt[:], xins[q][:, kc * P:(kc + 1) * P], ident[:])
                    nc.scalar.copy(xtt[:, kc, q * P:(q + 1) * P], tpt[:])
            xft = xf_pool.tile([P, HC, TOK], F32, tag="xf")
            xst = xs_pool.tile([P, HC, TOK], F32, tag="xs")
            for w_sb, b_t, dst in ((wfx, bft, xft), (wsx, bst, xst)):
                for jc in range(HC):
                    p1t = p1_pool.tile([P, TOK], F32, tag="p1")
                    for kc in range(ICC):
                        nc.tensor.matmul(
                            p1t[:], w_sb[:, kc, jc * P:(jc + 1) * P], xtt[:, kc, :],
                            start=(kc == 0), stop=(kc == ICC - 1))
                    nc.scalar.activation(dst[:, jc, :], p1t[:], AF.Identity,
                                         bias=b_t[:, jc:jc + 1], scale=1.0)
            return xft, xst

        HH = HC // 2

        def mk_gate_psums(pa_pool, pb_pool, ta, tb):
            return (pa_pool.tile([P, HH, B], F32, tag=ta, name=ta),
                    pb_pool.tile([P, HH, B], F32, tag=tb, name=tb))

        def views(c, xft, xst, htt):
            return tuple(t[:].rearrange("p h (q b t) -> p h q t b", b=B, t=TL)
                         for t in (xft, xst, htt))

        def gate_mm(dst_ab, w_sb, rhs4):
            """16 accumulating matmuls kc-outer into the split psum halves.
            rhs4 is a [P, HC, B] AP supplier: rhs4(kc) -> [P, B]."""
            for kc in range(HC):
                for jc in range(HC):
                    dst = dst_ab[0] if jc < HH else dst_ab[1]
                    nc.tensor.matmul(
                        dst[:, jc % HH, :], w_sb[:, kc, jc * P:(jc + 1) * P],
                        rhs4(kc), start=False, stop=True, skip_group_check=True)

        # Init the four scan psum banks: one zero matmul each (start=True)
        # sets the has_written bits so later start=False matmuls accumulate
        # onto ScalarE-preloaded data instead of overwriting it.
        init_pf = mk_gate_psums(pfa_pool, pfb_pool, "pfa", "pfb")
        init_ps = mk_gate_psums(psa_pool, psb_pool, "psa", "psb")
        for pt in (*init_pf, *init_ps):
            nc.tensor.matmul(pt[:].rearrange("p a b -> p (a b)"), ident[:],
                             h0[:, 0:2, :].rearrange("p a b -> p (a b)"),
                             start=True, stop=True)

        def scan_step(gt, cviews, nviews, hprev, cur, nxt_exists):
            """One recurrent step. cur = (pf_a, pf_b, ps_a, ps_b) holding
            zf(t) (= Xf + W.t1 + W.m accumulated by the previous step) and
            Xs(t). Emits this step's gate math plus the preload + W.t1 + W.m
            accumulation into the next step's psum tiles. Returns (hnew, nxt).
            """
            c, t = divmod(gt, TC)
            q, tl = t // TL, t % TL
            xft_v, xst_v, htt_v = cviews
            hnew = htt_v[:, :, q, tl, :]

            if gt == 0:
                # h0 = 0: f = sigmoid(Xf), s = tanh(Xs), h1 = f*s (= m; t1 = 0)
                f = sc_pool.tile([P, HC, B], F32, tag="f")
                nc.scalar.activation(f[:], xft_v[:, :, q, tl, :], AF.Sigmoid)
                s = sc_pool.tile([P, HC, B], F32, tag="s")
                nc.scalar.activation(s[:], xst_v[:, :, q, tl, :], AF.Tanh)
                m = sc_pool.tile([P, HC, B], BF16, tag="m")
                nc.vector.tensor_mul(m[:], f[:], s[:])
                nc.vector.tensor_copy(hnew, m[:])
                t1 = None
            else:
                pf_a, pf_b, ps_a, ps_b = cur
                f = sc_pool.tile([P, HC, B], F32, tag="f")
                nc.scalar.activation(f[:, 0:HH, :], pf_a[:], AF.Sigmoid)
                nc.scalar.activation(f[:, HH:HC, :], pf_b[:], AF.Sigmoid)

            # preload next step's Xf (ScalarE, after this step's sigmoid reads)
            nxt = None
            if nxt_exists:
                nxf_v, nxs_v, _ = nviews
                ngt = gt + 1
                nq, ntl = (ngt % TC) // TL, (ngt % TC) % TL
                npf = mk_gate_psums(pfa_pool, pfb_pool, "pfa", "pfb")
                nc.scalar.copy(npf[0][:], nxf_v[:, 0:HH, nq, ntl, :])
                nc.scalar.copy(npf[1][:], nxf_v[:, HH:HC, nq, ntl, :])

            if gt > 0:
                g = sc_pool.tile([P, HC, B], BF16, tag="g")
                t1 = sc_pool.tile([P, HC, B], BF16, tag="t1")
                for h_ in range(2):
                    lo, hi = h_ * HH, (h_ + 1) * HH
                    nc.vector.tensor_mul(g[:, lo:hi, :], f[:, lo:hi, :],
                                         hprev[:, lo:hi, :])
                for h_ in range(2):
                    lo, hi = h_ * HH, (h_ + 1) * HH
                    nc.vector.tensor_sub(t1[:, lo:hi, :], hprev[:, lo:hi, :],
                                         g[:, lo:hi, :])
                gate_mm((ps_a, ps_b), wsh, lambda kc: g[:, kc, :])
                s = sc_pool.tile([P, HC, B], F32, tag="s")
                nc.scalar.activation(s[:, 0:HH, :], ps_a[:], AF.Tanh)
                nc.scalar.activation(s[:, HH:HC, :], ps_b[:], AF.Tanh)

            if nxt_exists:
                nps = mk_gate_psums(psa_pool, psb_pool, "psa", "psb")
                nc.scalar.copy(nps[0][:], nxs_v[:, 0:HH, nq, ntl, :])
                nc.scalar.copy(nps[1][:], nxs_v[:, HH:HC, nq, ntl, :])
                nxt = (*npf, *nps)

            if gt > 0:
                m = sc_pool.tile([P, HC, B], BF16, tag="m")
                for h_ in range(2):
                    lo, hi = h_ * HH, (h_ + 1) * HH
                    nc.vector.tensor_mul(m[:, lo:hi, :], f[:, lo:hi, :],
                                         s[:, lo:hi, :])

            # fold the state update into the NEXT step's zf accumulation:
            # zf(t+1) = Xf(t+1) + W.t1(t) + W.m(t);  h' = t1 + m is computed
            # off the critical path (only feeds g(t+1), phase3 and hfin).
            if nxt_exists:
                if t1 is not None:
                    gate_mm(npf, wfh, lambda kc: t1[:, kc, :])
                gate_mm(npf, wfh, lambda kc: m[:, kc, :])

            if gt > 0:
                for h_ in range(2):
                    lo, hi = h_ * HH, (h_ + 1) * HH
                    nc.vector.tensor_add(hnew[:, lo:hi, :], t1[:, lo:hi, :],
                                         m[:, lo:hi, :])
            return hnew, nxt

        def phase3(c, htt):
            """Output projection + relu for chunk c, DMA to out."""
            for q in range(QN):
                pot = po_pool.tile([P, O], F32, tag="po")
                for kc in range(HC):
                    nc.tensor.matmul(pot[:], htt[:, kc, q * P:(q + 1) * P],
                                     wo_sb[:, kc, :], start=(kc == 0),
                                     stop=(kc == HC - 1))
                ob = ob_pool.tile([P, O], F32, tag="ob")
                nc.vector.tensor_add(ob[:], pot[:], bo_bc[:])
                ob2 = ob_pool.tile([P, O], F32, tag="ob2")
                nc.vector.tensor_scalar_max(ob2[:], ob[:], 0.0)
                t0 = c * TC + q * TL
                t1 = c * TC + (q + 1) * TL
                for b in range(B):
                    dma(out_d[b, t0:t1, :], ob2[b * TL:(b + 1) * TL, :])

        hprev = h0[:, :, :]
        chunk_data = {}

        def get_chunk(c):
            if c not in chunk_data and c < NCH:
                xft, xst = phase1(c)
                htt = ht_pool.tile([P, HC, TOK], BF16, tag="ht")
                chunk_data[c] = (xft, xst, htt, views(c, xft, xst, htt))
            return chunk_data.get(c)

        cur = init_pf + init_ps
        get_chunk(0)
        get_chunk(1)
        for c in range(NCH):
            _, _, htt, cviews = chunk_data[c]
            for t in range(TC):
                gt = c * TC + t
                nxt_exists = gt + 1 < seq_len
                nch_ = (gt + 1) // TC
                nviews = chunk_data[nch_][3] if nxt_exists else None
                hprev, nxt = scan_step(gt, cviews, nviews, hprev, cur, nxt_exists)
                cur = nxt
            phase3(c, htt)
            del chunk_data[c]
            get_chunk(c + 2)

        # final state: transpose h_last [P, HC, B] (bf16) -> [B, H] fp32
        for kc in range(HC):
            tpt = tp_pool.tile([P, P], BF16, tag="tp")
            nc.tensor.transpose(tpt[:B, :], hprev[:, kc, :], ident[:])
            nc.vector.tensor_copy(hfin_sb[:, kc * P:(kc + 1) * P], tpt[:B, :])
        dma(hfin_d[:, :], hfin_sb[:])

    nc.compile()
    return nc


_NC_CACHE = {}


def _get_nc():
    if "nc" not in _NC_CACHE:
        _NC_CACHE["nc"] = build()
    return _NC_CACHE["nc"]


def kernel(inputs, Wf, bf, Ws, bs, Wo, bo):
    from concourse.bass_utils import run_bass_kernel_spmd

    inputs = np.ascontiguousarray(np.asarray(inputs, dtype=np.float32))
    Wf = np.ascontiguousarray(np.asarray(Wf, dtype=np.float32))
    bf = np.ascontiguousarray(np.asarray(bf, dtype=np.float32))
    Ws = np.ascontiguousarray(np.asarray(Ws, dtype=np.float32))
    bs = np.ascontiguousarray(np.asarray(bs, dtype=np.float32))
    Wo = np.ascontiguousarray(np.asarray(Wo, dtype=np.float32))
    bo = np.ascontiguousarray(np.asarray(bo, dtype=np.float32))

    nc = _get_nc()
    batch = inputs.shape[0]
    bpc = batch // N_CORES
    in_maps = []
    for i in range(N_CORES):
        in_maps.append({
            "x": inputs[i * bpc:(i + 1) * bpc],
            "Wf": Wf, "bf": bf, "Ws": Ws, "bs": bs, "Wo": Wo, "bo": bo,
        })
    res = run_bass_kernel_spmd(
        nc, in_maps, core_ids=list(range(N_CORES)),
        trace=bool(int(os.environ.get("KERNEL_TRACE", "0"))),
    )
    outs = np.concatenate([r["out"] for r in res.results], axis=0)
    hfin = np.concatenate([r["hfin"] for r in res.results], axis=0)
    _NC_CACHE["last_results"] = res
    return outs, hfin


# revision 31
# speedup vs baseline: 5.8834x; 1.0012x over previous
"""MGU (minimal gated unit) RNN kernel for Trainium2, 8-core data-parallel.

Problem (hardcoded):
    inputs [64, 512, 256] f32, Wf/Ws [768, 512], bf/bs [512], Wo [512, 256], bo [256]
    h_t = (1-f)*h + f*tanh(([f*h, x] @ Ws) + bs),  f = sigmoid(([h, x] @ Wf) + bf)
    outputs = relu(all_hidden @ Wo + bo);  returns (outputs, final_state)

Sharding: batch 64 -> 8 cores x 8. Weights replicated. Scan local per core.

Layout: "transposed-land" — hidden on partitions, (time, batch) on the free dim.
    hidden index h = jc*128 + p; within a 64-step chunk, token col = q*128 + b*16 + t
Scan matmuls are weights-stationary bf16 (fp32 runs as a 4-cycle/row double
pass on this PE; bf16 LDW+MM pairs measure ~42ns vs fp32 ~435ns):
    matmul(psum[:, jc, :], lhsT=Wh_bf[:, kc, jc*128:+128], rhs=h_bf[:, kc, :])
Gate math runs fp32 from PSUM; the hidden state is stored bf16 (matmul rhs +
output-projection stationary). Input/output projections are wide-N bf16
matmuls per 512-token chunk, fully fused (no DRAM scratch).
"""

import os
import sys
import types
from contextlib import ExitStack

import numpy as np

# The agent image's `antenv` package lacks `axon_hooks`, which degrades NTFF
# profiling (trace=True) to a hard ImportError in bass_utils. Pre-seed a
# minimal implementation and register the ctypes-driven hook.
if "antenv.axon_hooks" not in sys.modules:
    _m = types.ModuleType("antenv.axon_hooks")
    _m._hook = None

    def _set_hook(h, _m=_m):
        _m._hook = h

    def _get_hook(_m=_m):
        return _m._hook

    _m.set_axon_ntff_profile_hook = _set_hook
    _m.get_axon_ntff_profile_hook = _get_hook
    sys.modules["antenv.axon_hooks"] = _m
    try:
        from trn_agent_boot.trn_boot import _ntff_profile_via_ctypes

        _h = _ntff_profile_via_ctypes("/opt/axon/libaxon_pjrt.so")
        if _h is not None:
            _set_hook(_h)
    except Exception:
        pass

import concourse.bacc as bacc
import concourse.mybir as mybir
from concourse import masks, tile

P = 128
B = 8          # batch per core
S = 512        # sequence length
IN = 256       # input units
H = 512        # hidden units
O = 256        # output units
HC = H // P    # hidden chunks (4)
ICC = IN // P  # input chunks (2)
N_CORES = 8
F32 = mybir.dt.float32
BF16 = mybir.dt.bfloat16


def build(seq_len=S, debug=False):
    nc = bacc.Bacc(None, target_bir_lowering=False, debug=debug)
    AF = mybir.ActivationFunctionType

    TC = min(64, seq_len)          # timesteps per chunk
    NCH = seq_len // TC            # chunks
    TOK = TC * B                   # tokens per chunk (<= 512)
    QN = TOK // P                  # 128-token subtiles per chunk
    TL = P // B                    # t_locals per subtile (16)

    x_d = nc.dram_tensor("x", [B, seq_len, IN], F32, kind="ExternalInput")
    wf_d = nc.dram_tensor("Wf", [H + IN, H], F32, kind="ExternalInput")
    bf_d = nc.dram_tensor("bf", [H], F32, kind="ExternalInput")
    ws_d = nc.dram_tensor("Ws", [H + IN, H], F32, kind="ExternalInput")
    bs_d = nc.dram_tensor("bs", [H], F32, kind="ExternalInput")
    wo_d = nc.dram_tensor("Wo", [H, O], F32, kind="ExternalInput")
    bo_d = nc.dram_tensor("bo", [O], F32, kind="ExternalInput")
    out_d = nc.dram_tensor("out", [B, seq_len, O], F32, kind="ExternalOutput")
    hfin_d = nc.dram_tensor("hfin", [B, H], F32, kind="ExternalOutput")

    with tile.TileContext(nc) as tc, ExitStack() as ctx:
        dma = nc.sync.dma_start
        cpool = ctx.enter_context(tc.tile_pool(name="const", bufs=1))

        # --- weights/constants in SBUF (bf16 compute copies) ---
        wfh = cpool.tile([P, HC, H], BF16)
        wsh = cpool.tile([P, HC, H], BF16)
        wfx = cpool.tile([P, ICC, H], BF16)
        wsx = cpool.tile([P, ICC, H], BF16)
        wo_sb = cpool.tile([P, HC, O], BF16)
        bft = cpool.tile([P, HC], F32)
        bst = cpool.tile([P, HC], F32)
        ident = cpool.tile([P, P], BF16)
        h0 = cpool.tile([P, HC, B], BF16)
        ones = cpool.tile([1, P], F32)
        bo_row = cpool.tile([1, O], F32)
        bo_bc = cpool.tile([P, O], F32)
        hfin_sb = cpool.tile([B, H], F32)
        stg = cpool.tile([P, HC, H], F32)   # fp32 staging for weight casts

        def load_cast(dst, src_ap, nch):
            dma(stg[:, :nch, :dst.shape[2]], src_ap)
            nc.vector.tensor_copy(dst[:], stg[:, :nch, :dst.shape[2]])

        load_cast(wfh, wf_d[0:H, :].rearrange("(kc p) j -> p kc j", p=P), HC)
        load_cast(wsh, ws_d[0:H, :].rearrange("(kc p) j -> p kc j", p=P), HC)
        load_cast(wfx, wf_d[H:H + IN, :].rearrange("(kc p) j -> p kc j", p=P), ICC)
        load_cast(wsx, ws_d[H:H + IN, :].rearrange("(kc p) j -> p kc j", p=P), ICC)
        load_cast(wo_sb, wo_d.rearrange("(kc p) o -> p kc o", p=P), HC)
        dma(bft[:], bf_d.rearrange("(jc p) -> p jc", p=P))
        dma(bst[:], bs_d.rearrange("(jc p) -> p jc", p=P))
        dma(bo_row[:], bo_d[None, :])
        masks.make_identity(nc, ident[:])
        nc.vector.memset(ones[:], 1.0)
        nc.vector.memset(h0[:], 0.0)

        # psum pools (8 banks: 1+1+1+1+1+1+2). The f/s gate accumulators are
        # split into per-half banks so sigmoid/tanh of the first half can run
        # while the PE still writes the second half's bank (the PSUM bank
        # rule serializes any reader behind all same-bank PE writes).
        tp_pool = ctx.enter_context(tc.tile_pool(name="tp", bufs=1, space="PSUM"))
        p1_pool = ctx.enter_context(tc.tile_pool(name="p1", bufs=1, space="PSUM"))
        pfa_pool = ctx.enter_context(tc.tile_pool(name="pfa", bufs=1, space="PSUM"))
        pfb_pool = ctx.enter_context(tc.tile_pool(name="pfb", bufs=1, space="PSUM"))
        psa_pool = ctx.enter_context(tc.tile_pool(name="psa", bufs=1, space="PSUM"))
        psb_pool = ctx.enter_context(tc.tile_pool(name="psb", bufs=1, space="PSUM"))
        po_pool = ctx.enter_context(tc.tile_pool(name="po", bufs=2, space="PSUM"))

        # bo broadcast across partitions via ones-matmul
        pbo = po_pool.tile([P, O], F32, tag="po")
        nc.tensor.matmul(pbo[:], ones[:], bo_row[:], start=True, stop=True)
        nc.vector.tensor_copy(bo_bc[:], pbo[:])

        # sbuf pools
        xin_pool = ctx.enter_context(tc.tile_pool(name="xin", bufs=2 * QN))
        xt_pool = ctx.enter_context(tc.tile_pool(name="xt", bufs=2))
        xf_pool = ctx.enter_context(tc.tile_pool(name="xf", bufs=2))
        xs_pool = ctx.enter_context(tc.tile_pool(name="xs", bufs=2))
        ht_pool = ctx.enter_context(tc.tile_pool(name="ht", bufs=2))
        sc_pool = ctx.enter_context(tc.tile_pool(name="sc", bufs=3))
        ob_pool = ctx.enter_context(tc.tile_pool(name="ob", bufs=3))

        def phase1(c):
            """Input projections for chunk c -> (XfT, XsT) [P, HC, TOK] fp32."""
            xins = []
            for q in range(QN):
                xi = xin_pool.tile([P, IN], F32, tag="xin")
                xib = xin_pool.tile([P, IN], BF16, tag="xib")
                t0 = c * TC + q * TL
                t1 = c * TC + (q + 1) * TL
                for b in range(B):
                    dma(xi[b * TL:(b + 1) * TL, :], x_d[b, t0:t1, :])
                nc.vector.tensor_copy(xib[:], xi[:])
                xins.append(xib)
            xtt = xt_pool.tile([P, ICC, TOK], BF16, tag="xt")
            for q in range(QN):
                for kc in range(ICC):
                    tpt = tp_pool.tile([P, P], BF16, tag="tp")
                    nc.tensor.transpose(tpt[:], xins[q][:, kc * P:(kc + 1) * P], ident[:])
                    nc.scalar.copy(xtt[:, kc, q * P:(q + 1) * P], tpt[:])
            xft = xf_pool.tile([P, HC, TOK], F32, tag="xf")
            xst = xs_pool.tile([P, HC, TOK], F32, tag="xs")
            for w_sb, b_t, dst in ((wfx, bft, xft), (wsx, bst, xst)):
                for jc in range(HC):
                    p1t = p1_pool.tile([P, TOK], F32, tag="p1")
                    for kc in range(ICC):
                        nc.tensor.matmul(
                            p1t[:], w_sb[:, kc, jc * P:(jc + 1) * P], xtt[:, kc, :],
                            start=(kc == 0), stop=(kc == ICC - 1))
                    nc.scalar.activation(dst[:, jc, :], p1t[:], AF.Identity,
                                         bias=b_t[:, jc:jc + 1], scale=1.0)
            return xft, xst

        HH = HC // 2

        def mk_gate_psums(pa_pool, pb_pool, ta, tb):
            return (pa_pool.tile([P, HH, B], F32, tag=ta, name=ta),
                    pb_pool.tile([P, HH, B], F32, tag=tb, name=tb))

        def views(c, xft, xst, htt):
            return tuple(t[:].rearrange("p h (q b t) -> p h q t b", b=B, t=TL)
                         for t in (xft, xst, htt))

        def gate_mm(dst_ab, w_sb, rhs4):
            """16 accumulating matmuls kc-outer into the split psum halves.
            rhs4 is a [P, HC, B] AP supplier: rhs4(kc) -> [P, B]."""
            for kc in range(HC):
                for jc in range(HC):
                    dst = dst_ab[0] if jc < HH else dst_ab[1]
                    nc.tensor.matmul(
                        dst[:, jc % HH, :], w_sb[:, kc, jc * P:(jc + 1) * P],
                        rhs4(kc), start=False, stop=True, skip_group_check=True)

        # Init the four scan psum banks: one zero matmul each (start=True)
        # sets the has_written bits so later start=False matmuls accumulate
        # onto ScalarE-preloaded data instead of overwriting it.
        init_pf = mk_gate_psums(pfa_pool, pfb_pool, "pfa", "pfb")
        init_ps = mk_gate_psums(psa_pool, psb_pool, "psa", "psb")
        for pt in (*init_pf, *init_ps):
            nc.tensor.matmul(pt[:].rearrange("p a b -> p (a b)"), ident[:],
                             h0[:, 0:2, :].rearrange("p a b -> p (a b)"),
                             start=True, stop=True)

        def scan_step(gt, cviews, nviews, hprev, cur, nxt_exists):
            """One recurrent step. cur = (pf_a, pf_b, ps_a, ps_b) holding
            zf(t) (= Xf + W.t1 + W.m accumulated by the previous step) and
            Xs(t). Emits this step's gate math plus the preload + W.t1 + W.m
            accumulation into the next step's psum tiles. Returns (hnew, nxt).
            """
            c, t = divmod(gt, TC)
            q, tl = t // TL, t % TL
            xft_v, xst_v, htt_v = cviews
            hnew = htt_v[:, :, q, tl, :]

            if gt == 0:
                # h0 = 0: f = sigmoid(Xf), s = tanh(Xs), h1 = f*s (= m; t1 = 0)
                f = sc_pool.tile([P, HC, B], F32, tag="f")
                nc.scalar.activation(f[:], xft_v[:, :, q, tl, :], AF.Sigmoid)
                s = sc_pool.tile([P, HC, B], F32, tag="s")
                nc.scalar.activation(s[:], xst_v[:, :, q, tl, :], AF.Tanh)
                m = sc_pool.tile([P, HC, B], BF16, tag="m")
                nc.vector.tensor_mul(m[:], f[:], s[:])
                nc.vector.tensor_copy(hnew, m[:])
                t1 = None
            else:
                pf_a, pf_b, ps_a, ps_b = cur
                f = sc_pool.tile([P, HC, B], F32, tag="f")
                nc.scalar.activation(f[:, 0:HH, :], pf_a[:], AF.Sigmoid)
                nc.scalar.activation(f[:, HH:HC, :], pf_b[:], AF.Sigmoid)

            # preload next step's Xf (ScalarE, after this step's sigmoid reads)
            nxt = None
            if nxt_exists:
                nxf_v, nxs_v, _ = nviews
                ngt = gt + 1
                nq, ntl = (ngt % TC) // TL, (ngt % TC) % TL
                npf = mk_gate_psums(pfa_pool, pfb_pool, "pfa", "pfb")
                nc.scalar.copy(npf[0][:], nxf_v[:, 0:HH, nq, ntl, :])
                nc.scalar.copy(npf[1][:], nxf_v[:, HH:HC, nq, ntl, :])

            if gt > 0:
                g = sc_pool.tile([P, HC, B], BF16, tag="g")
                t1 = sc_pool.tile([P, HC, B], BF16, tag="t1")
                for h_ in range(2):
                    lo, hi = h_ * HH, (h_ + 1) * HH
                    nc.vector.tensor_mul(g[:, lo:hi, :], f[:, lo:hi, :],
                                         hprev[:, lo:hi, :])
                for h_ in range(2):
                    lo, hi = h_ * HH, (h_ + 1) * HH
                    nc.vector.tensor_sub(t1[:, lo:hi, :], hprev[:, lo:hi, :],
                                         g[:, lo:hi, :])
                gate_mm((ps_a, ps_b), wsh, lambda kc: g[:, kc, :])
                s = sc_pool.tile([P, HC, B], F32, tag="s")
                nc.scalar.activation(s[:, 0:HH, :], ps_a[:], AF.Tanh)
                nc.scalar.activation(s[:, HH:HC, :], ps_b[:], AF.Tanh)

            if nxt_exists:
                nps = mk_gate_psums(psa_pool, psb_pool, "psa", "psb")
                nc.scalar.copy(nps[0][:], nxs_v[:, 0:HH, nq, ntl, :])
                nc.scalar.copy(nps[1][:], nxs_v[:, HH:HC, nq, ntl, :])
                nxt = (*npf, *nps)

            if gt > 0:
                m = sc_pool.tile([P, HC, B], BF16, tag="m")
                for h_ in range(2):
                    lo, hi = h_ * HH, (h_ + 1) * HH
                    nc.vector.tensor_mul(m[:, lo:hi, :], f[:, lo:hi, :],
                                         s[:, lo:hi, :])

            # fold the state update into the NEXT step's zf accumulation:
            # zf(t+1) = Xf(t+1) + W.t1(t) + W.m(t);  h' = t1 + m is computed
            # off the critical path (only feeds g(t+1), phase3 and hfin).
            if nxt_exists:
                if t1 is not None:
                    gate_mm(npf, wfh, lambda kc: t1[:, kc, :])
                gate_mm(npf, wfh, lambda kc: m[:, kc, :])

            if gt > 0:
                for h_ in range(2):
                    lo, hi = h_ * HH, (h_ + 1) * HH
                    nc.vector.tensor_add(hnew[:, lo:hi, :], t1[:, lo:hi, :],
                                         m[:, lo:hi, :])
            return hnew, nxt

        def phase3(c, htt):
            """Output projection + relu for chunk c, DMA to out."""
            for q in range(QN):
                pot = po_pool.tile([P, O], F32, tag="po")
                for kc in range(HC):
                    nc.tensor.matmul(pot[:], htt[:, kc, q * P:(q + 1) * P],
                                     wo_sb[:, kc, :], start=(kc == 0),
                                     stop=(kc == HC - 1))
                ob = ob_pool.tile([P, O], F32, tag="ob")
                nc.vector.tensor_add(ob[:], pot[:], bo_bc[:])
                ob2 = ob_pool.tile([P, O], F32, tag="ob2")
                nc.vector.tensor_scalar_max(ob2[:], ob[:], 0.0)
                t0 = c * TC + q * TL
                t1 = c * TC + (q + 1) * TL
                for b in range(B):
                    dma(out_d[b, t0:t1, :], ob2[b * TL:(b + 1) * TL, :])

        hprev = h0[:, :, :]
        chunk_data = {}

        def get_chunk(c):
            if c not in chunk_data and c < NCH:
                xft, xst = phase1(c)
                htt = ht_pool.tile([P, HC, TOK], BF16, tag="ht")
                chunk_data[c] = (xft, xst, htt, views(c, xft, xst, htt))
            return chunk_data.get(c)

        cur = init_pf + init_ps
        get_chunk(0)
        get_chunk(1)
        for c in range(NCH):
            _, _, htt, cviews = chunk_data[c]
            for t in range(TC):
                gt = c * TC + t
                nxt_exists = gt + 1 < seq_len
                nch_ = (gt + 1) // TC
                nviews = chunk_data[nch_][3] if nxt_exists else None
                hprev, nxt = scan_step(gt, cviews, nviews, hprev, cur, nxt_exists)
                cur = nxt
            phase3(c, htt)
            del chunk_data[c]
            get_chunk(c + 2)

        # final state: transpose h_last [P, HC, B] (bf16) -> [B, H] fp32
        for kc in range(HC):
            tpt = tp_pool.tile([P, P], BF16, tag="tp")
            nc.tensor.transpose(tpt[:B, :], hprev[:, kc, :], ident[:])
            nc.vector.tensor_copy(hfin_sb[:, kc * P:(kc + 1) * P], tpt[:B, :])
        dma(hfin_d[:, :], hfin_sb[:])

    nc.compile()
    return nc


_NC_CACHE = {}


def _get_nc():
    if "nc" not in _NC_CACHE:
        _NC_CACHE["nc"] = build()
    return _NC_CACHE["nc"]


def kernel(inputs, Wf, bf, Ws, bs, Wo, bo):
    from concourse.bass_utils import run_bass_kernel_spmd

    inputs = np.ascontiguousarray(np.asarray(inputs, dtype=np.float32))
    Wf = np.ascontiguousarray(np.asarray(Wf, dtype=np.float32))
    bf = np.ascontiguousarray(np.asarray(bf, dtype=np.float32))
    Ws = np.ascontiguousarray(np.asarray(Ws, dtype=np.float32))
    bs = np.ascontiguousarray(np.asarray(bs, dtype=np.float32))
    Wo = np.ascontiguousarray(np.asarray(Wo, dtype=np.float32))
    bo = np.ascontiguousarray(np.asarray(bo, dtype=np.float32))

    nc = _get_nc()
    batch = inputs.shape[0]
    bpc = batch // N_CORES
    in_maps = []
    for i in range(N_CORES):
        in_maps.append({
            "x": inputs[i * bpc:(i + 1) * bpc],
            "Wf": Wf, "bf": bf, "Ws": Ws, "bs": bs, "Wo": Wo, "bo": bo,
        })
    res = run_bass_kernel_spmd(
        nc, in_maps, core_ids=list(range(N_CORES)),
        trace=bool(int(os.environ.get("KERNEL_TRACE", "0"))),
    )
    outs = np.concatenate([r["out"] for r in res.results], axis=0)
    hfin = np.concatenate([r["hfin"] for r in res.results], axis=0)
    _NC_CACHE["last_results"] = res
    return outs, hfin


# revision 32
# speedup vs baseline: 5.8999x; 1.0028x over previous
"""MGU (minimal gated unit) RNN kernel for Trainium2, 8-core data-parallel.

Problem (hardcoded):
    inputs [64, 512, 256] f32, Wf/Ws [768, 512], bf/bs [512], Wo [512, 256], bo [256]
    h_t = (1-f)*h + f*tanh(([f*h, x] @ Ws) + bs),  f = sigmoid(([h, x] @ Wf) + bf)
    outputs = relu(all_hidden @ Wo + bo);  returns (outputs, final_state)

Sharding: batch 64 -> 8 cores x 8. Weights replicated. Scan local per core.

Layout: "transposed-land" — hidden on partitions, (time, batch) on the free dim.
    hidden index h = jc*128 + p; within a 64-step chunk, token col = q*128 + b*16 + t
Scan matmuls are weights-stationary bf16 (fp32 runs as a 4-cycle/row double
pass on this PE; bf16 LDW+MM pairs measure ~42ns vs fp32 ~435ns):
    matmul(psum[:, jc, :], lhsT=Wh_bf[:, kc, jc*128:+128], rhs=h_bf[:, kc, :])
Gate math runs fp32 from PSUM; the hidden state is stored bf16 (matmul rhs +
output-projection stationary). Input/output projections are wide-N bf16
matmuls per 512-token chunk, fully fused (no DRAM scratch).
"""

import os
import sys
import types
from contextlib import ExitStack

import numpy as np

# The agent image's `antenv` package lacks `axon_hooks`, which degrades NTFF
# profiling (trace=True) to a hard ImportError in bass_utils. Pre-seed a
# minimal implementation and register the ctypes-driven hook.
if "antenv.axon_hooks" not in sys.modules:
    _m = types.ModuleType("antenv.axon_hooks")
    _m._hook = None

    def _set_hook(h, _m=_m):
        _m._hook = h

    def _get_hook(_m=_m):
        return _m._hook

    _m.set_axon_ntff_profile_hook = _set_hook
    _m.get_axon_ntff_profile_hook = _get_hook
    sys.modules["antenv.axon_hooks"] = _m
    try:
        from trn_agent_boot.trn_boot import _ntff_profile_via_ctypes

        _h = _ntff_profile_via_ctypes("/opt/axon/libaxon_pjrt.so")
        if _h is not None:
            _set_hook(_h)
    except Exception:
        pass

import concourse.bacc as bacc
import concourse.mybir as mybir
from concourse import masks, tile

P = 128
B = 8          # batch per core
S = 512        # sequence length
IN = 256       # input units
H = 512        # hidden units
O = 256        # output units
HC = H // P    # hidden chunks (4)
ICC = IN // P  # input chunks (2)
N_CORES = 8
F32 = mybir.dt.float32
BF16 = mybir.dt.bfloat16


def build(seq_len=S, debug=False):
    nc = bacc.Bacc(None, target_bir_lowering=False, debug=debug)
    AF = mybir.ActivationFunctionType

    TC = min(64, seq_len)          # timesteps per chunk
    NCH = seq_len // TC            # chunks
    TOK = TC * B                   # tokens per chunk (<= 512)
    QN = TOK // P                  # 128-token subtiles per chunk
    TL = P // B                    # t_locals per subtile (16)

    x_d = nc.dram_tensor("x", [B, seq_len, IN], F32, kind="ExternalInput")
    wf_d = nc.dram_tensor("Wf", [H + IN, H], F32, kind="ExternalInput")
    bf_d = nc.dram_tensor("bf", [H], F32, kind="ExternalInput")
    ws_d = nc.dram_tensor("Ws", [H + IN, H], F32, kind="ExternalInput")
    bs_d = nc.dram_tensor("bs", [H], F32, kind="ExternalInput")
    wo_d = nc.dram_tensor("Wo", [H, O], F32, kind="ExternalInput")
    bo_d = nc.dram_tensor("bo", [O], F32, kind="ExternalInput")
    out_d = nc.dram_tensor("out", [B, seq_len, O], F32, kind="ExternalOutput")
    hfin_d = nc.dram_tensor("hfin", [B, H], F32, kind="ExternalOutput")

    with tile.TileContext(nc) as tc, ExitStack() as ctx:
        dma = nc.sync.dma_start
        cpool = ctx.enter_context(tc.tile_pool(name="const", bufs=1))

        # --- weights/constants in SBUF (bf16 compute copies) ---
        wfh = cpool.tile([P, HC, H], BF16)
        wsh = cpool.tile([P, HC, H], BF16)
        wfx = cpool.tile([P, ICC, H], BF16)
        wsx = cpool.tile([P, ICC, H], BF16)
        wo_sb = cpool.tile([P, HC, O], BF16)
        bft = cpool.tile([P, HC], F32)
        bst = cpool.tile([P, HC], F32)
        ident = cpool.tile([P, P], BF16)
        h0 = cpool.tile([P, HC, B], BF16)
        ones = cpool.tile([1, P], F32)
        bo_row = cpool.tile([1, O], F32)
        bo_bc = cpool.tile([P, O], F32)
        hfin_sb = cpool.tile([B, H], F32)
        stg = cpool.tile([P, HC, H], F32)   # fp32 staging for weight casts

        def load_cast(dst, src_ap, nch):
            dma(stg[:, :nch, :dst.shape[2]], src_ap)
            nc.vector.tensor_copy(dst[:], stg[:, :nch, :dst.shape[2]])

        load_cast(wfh, wf_d[0:H, :].rearrange("(kc p) j -> p kc j", p=P), HC)
        load_cast(wsh, ws_d[0:H, :].rearrange("(kc p) j -> p kc j", p=P), HC)
        load_cast(wfx, wf_d[H:H + IN, :].rearrange("(kc p) j -> p kc j", p=P), ICC)
        load_cast(wsx, ws_d[H:H + IN, :].rearrange("(kc p) j -> p kc j", p=P), ICC)
        load_cast(wo_sb, wo_d.rearrange("(kc p) o -> p kc o", p=P), HC)
        dma(bft[:], bf_d.rearrange("(jc p) -> p jc", p=P))
        dma(bst[:], bs_d.rearrange("(jc p) -> p jc", p=P))
        dma(bo_row[:], bo_d[None, :])
        masks.make_identity(nc, ident[:])
        nc.vector.memset(ones[:], 1.0)
        nc.vector.memset(h0[:], 0.0)

        # psum pools (8 banks: 1+1+1+1+1+1+2). The f/s gate accumulators are
        # split into per-half banks so sigmoid/tanh of the first half can run
        # while the PE still writes the second half's bank (the PSUM bank
        # rule serializes any reader behind all same-bank PE writes).
        tp_pool = ctx.enter_context(tc.tile_pool(name="tp", bufs=1, space="PSUM"))
        p1_pool = ctx.enter_context(tc.tile_pool(name="p1", bufs=1, space="PSUM"))
        pfa_pool = ctx.enter_context(tc.tile_pool(name="pfa", bufs=1, space="PSUM"))
        pfb_pool = ctx.enter_context(tc.tile_pool(name="pfb", bufs=1, space="PSUM"))
        psa_pool = ctx.enter_context(tc.tile_pool(name="psa", bufs=1, space="PSUM"))
        psb_pool = ctx.enter_context(tc.tile_pool(name="psb", bufs=1, space="PSUM"))
        po_pool = ctx.enter_context(tc.tile_pool(name="po", bufs=2, space="PSUM"))

        # bo broadcast across partitions via ones-matmul
        pbo = po_pool.tile([P, O], F32, tag="po")
        nc.tensor.matmul(pbo[:], ones[:], bo_row[:], start=True, stop=True)
        nc.vector.tensor_copy(bo_bc[:], pbo[:])

        # sbuf pools
        xin_pool = ctx.enter_context(tc.tile_pool(name="xin", bufs=2 * QN))
        xt_pool = ctx.enter_context(tc.tile_pool(name="xt", bufs=2))
        xf_pool = ctx.enter_context(tc.tile_pool(name="xf", bufs=2))
        xs_pool = ctx.enter_context(tc.tile_pool(name="xs", bufs=2))
        ht_pool = ctx.enter_context(tc.tile_pool(name="ht", bufs=2))
        sc_pool = ctx.enter_context(tc.tile_pool(name="sc", bufs=3))
        ob_pool = ctx.enter_context(tc.tile_pool(name="ob", bufs=3))

        def phase1(c):
            """Input projections for chunk c -> (XfT, XsT) [P, HC, TOK] fp32."""
            xins = []
            for q in range(QN):
                xi = xin_pool.tile([P, IN], F32, tag="xin")
                xib = xin_pool.tile([P, IN], BF16, tag="xib")
                t0 = c * TC + q * TL
                t1 = c * TC + (q + 1) * TL
                for b in range(B):
                    dma(xi[b * TL:(b + 1) * TL, :], x_d[b, t0:t1, :])
                nc.vector.tensor_copy(xib[:], xi[:])
                xins.append(xib)
            xtt = xt_pool.tile([P, ICC, TOK], BF16, tag="xt")
            for q in range(QN):
                for kc in range(ICC):
                    tpt = tp_pool.tile([P, P], BF16, tag="tp")
                    nc.tensor.transpose(tpt[:], xins[q][:, kc * P:(kc + 1) * P], ident[:])
                    nc.scalar.copy(xtt[:, kc, q * P:(q + 1) * P], tpt[:])
            xft = xf_pool.tile([P, HC, TOK], F32, tag="xf")
            xst = xs_pool.tile([P, HC, TOK], F32, tag="xs")
            for w_sb, b_t, dst in ((wfx, bft, xft), (wsx, bst, xst)):
                for jc in range(HC):
                    p1t = p1_pool.tile([P, TOK], F32, tag="p1")
                    for kc in range(ICC):
                        nc.tensor.matmul(
                            p1t[:], w_sb[:, kc, jc * P:(jc + 1) * P], xtt[:, kc, :],
                            start=(kc == 0), stop=(kc == ICC - 1))
                    nc.scalar.activation(dst[:, jc, :], p1t[:], AF.Identity,
                                         bias=b_t[:, jc:jc + 1], scale=1.0)
            return xft, xst

        HH = HC // 2

        def mk_gate_psums(pa_pool, pb_pool, ta, tb):
            return (pa_pool.tile([P, HH, B], F32, tag=ta, name=ta),
                    pb_pool.tile([P, HH, B], F32, tag=tb, name=tb))

        def views(c, xft, xst, htt):
            return tuple(t[:].rearrange("p h (q b t) -> p h q t b", b=B, t=TL)
                         for t in (xft, xst, htt))

        def gate_mm(dst_ab, w_sb, rhs4):
            """16 accumulating matmuls kc-outer into the split psum halves.
            rhs4 is a [P, HC, B] AP supplier: rhs4(kc) -> [P, B]."""
            for kc in range(HC):
                for jc in range(HC):
                    dst = dst_ab[0] if jc < HH else dst_ab[1]
                    nc.tensor.matmul(
                        dst[:, jc % HH, :], w_sb[:, kc, jc * P:(jc + 1) * P],
                        rhs4(kc), start=False, stop=True, skip_group_check=True)

        # Init the four scan psum banks: one zero matmul each (start=True)
        # sets the has_written bits so later start=False matmuls accumulate
        # onto ScalarE-preloaded data instead of overwriting it.
        init_pf = mk_gate_psums(pfa_pool, pfb_pool, "pfa", "pfb")
        init_ps = mk_gate_psums(psa_pool, psb_pool, "psa", "psb")
        for pt in (*init_pf, *init_ps):
            nc.tensor.matmul(pt[:].rearrange("p a b -> p (a b)"), ident[:],
                             h0[:, 0:2, :].rearrange("p a b -> p (a b)"),
                             start=True, stop=True)

        def scan_step(gt, cviews, nviews, hprev, cur, nxt_exists):
            """One recurrent step. cur = (pf_a, pf_b, ps_a, ps_b) holding
            zf(t) (= Xf + W.t1 + W.m accumulated by the previous step) and
            Xs(t). Emits this step's gate math plus the preload + W.t1 + W.m
            accumulation into the next step's psum tiles. Returns (hnew, nxt).
            """
            c, t = divmod(gt, TC)
            q, tl = t // TL, t % TL
            xft_v, xst_v, htt_v = cviews
            hnew = htt_v[:, :, q, tl, :]

            if gt == 0:
                # h0 = 0: f = sigmoid(Xf), s = tanh(Xs), h1 = f*s (= m; t1 = 0)
                f = sc_pool.tile([P, HC, B], BF16, tag="f")
                nc.scalar.activation(f[:], xft_v[:, :, q, tl, :], AF.Sigmoid)
                s = sc_pool.tile([P, HC, B], BF16, tag="s")
                nc.scalar.activation(s[:], xst_v[:, :, q, tl, :], AF.Tanh)
                m = sc_pool.tile([P, HC, B], BF16, tag="m")
                nc.vector.tensor_mul(m[:], f[:], s[:])
                nc.vector.tensor_copy(hnew, m[:])
                t1 = None
            else:
                pf_a, pf_b, ps_a, ps_b = cur
                f = sc_pool.tile([P, HC, B], BF16, tag="f")
                nc.scalar.activation(f[:, 0:HH, :], pf_a[:], AF.Sigmoid)
                nc.scalar.activation(f[:, HH:HC, :], pf_b[:], AF.Sigmoid)

            # preload next step's Xf (ScalarE, after this step's sigmoid reads)
            nxt = None
            if nxt_exists:
                nxf_v, nxs_v, _ = nviews
                ngt = gt + 1
                nq, ntl = (ngt % TC) // TL, (ngt % TC) % TL
                npf = mk_gate_psums(pfa_pool, pfb_pool, "pfa", "pfb")
                nc.scalar.copy(npf[0][:], nxf_v[:, 0:HH, nq, ntl, :])
                nc.scalar.copy(npf[1][:], nxf_v[:, HH:HC, nq, ntl, :])

            if gt > 0:
                g = sc_pool.tile([P, HC, B], BF16, tag="g")
                t1 = sc_pool.tile([P, HC, B], BF16, tag="t1")
                for h_ in range(2):
                    lo, hi = h_ * HH, (h_ + 1) * HH
                    nc.vector.tensor_mul(g[:, lo:hi, :], f[:, lo:hi, :],
                                         hprev[:, lo:hi, :])
                for h_ in range(2):
                    lo, hi = h_ * HH, (h_ + 1) * HH
                    nc.vector.tensor_sub(t1[:, lo:hi, :], hprev[:, lo:hi, :],
                                         g[:, lo:hi, :])
                gate_mm((ps_a, ps_b), wsh, lambda kc: g[:, kc, :])
                s = sc_pool.tile([P, HC, B], BF16, tag="s")
                nc.scalar.activation(s[:, 0:HH, :], ps_a[:], AF.Tanh)
                nc.scalar.activation(s[:, HH:HC, :], ps_b[:], AF.Tanh)

            if nxt_exists:
                nps = mk_gate_psums(psa_pool, psb_pool, "psa", "psb")
                nc.scalar.copy(nps[0][:], nxs_v[:, 0:HH, nq, ntl, :])
                nc.scalar.copy(nps[1][:], nxs_v[:, HH:HC, nq, ntl, :])
                nxt = (*npf, *nps)

            if gt > 0:
                m = sc_pool.tile([P, HC, B], BF16, tag="m")
                for h_ in range(2):
                    lo, hi = h_ * HH, (h_ + 1) * HH
                    nc.vector.tensor_mul(m[:, lo:hi, :], f[:, lo:hi, :],
                                         s[:, lo:hi, :])

            # fold the state update into the NEXT step's zf accumulation:
            # zf(t+1) = Xf(t+1) + W.t1(t) + W.m(t);  h' = t1 + m is computed
            # off the critical path (only feeds g(t+1), phase3 and hfin).
            if nxt_exists:
                if t1 is not None:
                    gate_mm(npf, wfh, lambda kc: t1[:, kc, :])
                gate_mm(npf, wfh, lambda kc: m[:, kc, :])

            if gt > 0:
                for h_ in range(2):
                    lo, hi = h_ * HH, (h_ + 1) * HH
                    nc.vector.tensor_add(hnew[:, lo:hi, :], t1[:, lo:hi, :],
                                         m[:, lo:hi, :])
            return hnew, nxt

        def phase3(c, htt):
            """Output projection + relu for chunk c, DMA to out."""
            for q in range(QN):
                pot = po_pool.tile([P, O], F32, tag="po")
                for kc in range(HC):
                    nc.tensor.matmul(pot[:], htt[:, kc, q * P:(q + 1) * P],
                                     wo_sb[:, kc, :], start=(kc == 0),
                                     stop=(kc == HC - 1))
                ob = ob_pool.tile([P, O], F32, tag="ob")
                nc.vector.tensor_add(ob[:], pot[:], bo_bc[:])
                ob2 = ob_pool.tile([P, O], F32, tag="ob2")
                nc.vector.tensor_scalar_max(ob2[:], ob[:], 0.0)
                t0 = c * TC + q * TL
                t1 = c * TC + (q + 1) * TL
                for b in range(B):
                    dma(out_d[b, t0:t1, :], ob2[b * TL:(b + 1) * TL, :])

        hprev = h0[:, :, :]
        chunk_data = {}

        def get_chunk(c):
            if c not in chunk_data and c < NCH:
                xft, xst = phase1(c)
                htt = ht_pool.tile([P, HC, TOK], BF16, tag="ht")
                chunk_data[c] = (xft, xst, htt, views(c, xft, xst, htt))
            return chunk_data.get(c)

        cur = init_pf + init_ps
        get_chunk(0)
        get_chunk(1)
        for c in range(NCH):
            _, _, htt, cviews = chunk_data[c]
            for t in range(TC):
                gt = c * TC + t
                nxt_exists = gt + 1 < seq_len
                nch_ = (gt + 1) // TC
                nviews = chunk_data[nch_][3] if nxt_exists else None
                hprev, nxt = scan_step(gt, cviews, nviews, hprev, cur, nxt_exists)
                cur = nxt
            phase3(c, htt)
            del chunk_data[c]
            get_chunk(c + 2)

        # final state: transpose h_last [P, HC, B] (bf16) -> [B, H] fp32
        for kc in range(HC):
            tpt = tp_pool.tile([P, P], BF16, tag="tp")
            nc.tensor.transpose(tpt[:B, :], hprev[:, kc, :], ident[:])
            nc.vector.tensor_copy(hfin_sb[:, kc * P:(kc + 1) * P], tpt[:B, :])
        dma(hfin_d[:, :], hfin_sb[:])

    nc.compile()
    return nc


_NC_CACHE = {}


def _get_nc():
    if "nc" not in _NC_CACHE:
        _NC_CACHE["nc"] = build()
    return _NC_CACHE["nc"]


def kernel(inputs, Wf, bf, Ws, bs, Wo, bo):
    from concourse.bass_utils import run_bass_kernel_spmd

    inputs = np.ascontiguousarray(np.asarray(inputs, dtype=np.float32))
    Wf = np.ascontiguousarray(np.asarray(Wf, dtype=np.float32))
    bf = np.ascontiguousarray(np.asarray(bf, dtype=np.float32))
    Ws = np.ascontiguousarray(np.asarray(Ws, dtype=np.float32))
    bs = np.ascontiguousarray(np.asarray(bs, dtype=np.float32))
    Wo = np.ascontiguousarray(np.asarray(Wo, dtype=np.float32))
    bo = np.ascontiguousarray(np.asarray(bo, dtype=np.float32))

    nc = _get_nc()
    batch = inputs.shape[0]
    bpc = batch // N_CORES
    in_maps = []
    for i in range(N_CORES):
        in_maps.append({
            "x": inputs[i * bpc:(i + 1) * bpc],
            "Wf": Wf, "bf": bf, "Ws": Ws, "bs": bs, "Wo": Wo, "bo": bo,
        })
    res = run_bass_kernel_spmd(
        nc, in_maps, core_ids=list(range(N_CORES)),
        trace=bool(int(os.environ.get("KERNEL_TRACE", "0"))),
    )
    outs = np.concatenate([r["out"] for r in res.results], axis=0)
    hfin = np.concatenate([r["hfin"] for r in res.results], axis=0)
    _NC_CACHE["last_results"] = res
    return outs, hfin
